# revision 2
# baseline (speedup 1.0000x reference)
"""TRN2 Bass kernel for nn_NNModelEx_63513976373928.

Math (per row x of X [B, 38]):
  h1  = relu(x @ W1.T + b1)                  [256]
  h2  = relu(h1 @ W2.T + b2)                 [256]
  out = h2 @ W3.T + b3                       [128]
  per target t in (incl, ecc, mm), ridx in (7, 9, 12):
    lin = out . lw_t + lb_t
    e   = (out . bew_t) * eps + beb_t        eps = x[0]
    y_t = bw_t * e * lin + bb_t + x[ridx]

Device strategy (pure data parallel, 8 cores x 32768 rows):
  - feature-on-partition layout: H1T/H2T [units, rows], rows chunked by 512
  - b1 folded into the L1 matmul via an augmented contraction row
  - L3 + heads folded: lin/p are dots of h2 with W3.T @ lw / W3.T @ bew
  - X pre-transposed and fp8-cast on host -> XT [39, 32768] per core
  - residual/eps/bias columns packed fp32 on host -> XRB [128, 64, 4, 9]

v3 engine/bank plan (cross-engine handoff bound):
  - PSUM: one unified ring pool of 7 one-bank [128,512] f32 tiles
    (4 allocs/chunk: h1pa, h1pb, h2pa, h2pb -> 1.75 chunks of WAR
    slack on every PE-write-after-cast-read edge) + 1 bank for heads
  - DVE: two h1 half casts/chunk ([128,512] PSUM->SBUF fp8) + one
    heads-PSUM->SBUF copy per 16-chunk batch
  - ACT: the two h2 half casts (per-half b2 bias forces 2 instrs)
  - Pool (GPSIMD): all epilogue arithmetic, SBUF-only
  - PE: heads(c-4) first (tiny MMs), L1(c) non-DR fp8 (K=40, FWL),
    L2(c-2) DoubleRow; CPB=16 cuts batch-boundary work
  - measured on HW: v2 223us(harness)/191us(R-slope) -> v3 ~155us
"""

import sys

for _p in ("/opt/trn_rl_repo", "/opt/trn_rl_repo/concourse"):
    if _p not in sys.path:
        sys.path.insert(0, _p)

import numpy as np
import ml_dtypes

BF16 = ml_dtypes.bfloat16

NCORES = 8
B = 262144
D = 38
DA = 39                     # contraction with bias row appended
ROWS = B // NCORES          # 32768 rows per core
CHUNK = 512                 # rows per chunk
NCHUNK = ROWS // CHUNK      # 64
CPB = 16                    # chunks per staging batch
NBATCH = NCHUNK // CPB      # 8

_NC_CACHE = {}


def _build_nc(repeat=1):
    from concourse import bass, bacc, tile
    from contextlib import nullcontext

    mybir = bass.mybir
    f32 = mybir.dt.float32
    f8 = mybir.dt.float8e4

    nc = bacc.Bacc(None, target_bir_lowering=False, debug=False)

    XT = nc.dram_tensor("XT", [40, ROWS], f8, kind="ExternalInput")
    XRB = nc.dram_tensor("XRB", [128, NCHUNK, 4, 9], f32, kind="ExternalInput")
    W1T = nc.dram_tensor("W1T", [40, 256], f8, kind="ExternalInput")
    # W2H packs W2 (cols 0:256, two output halves) + head vectors (256:262);
    # padded to 272 so the dim-1 stride is 16B-aligned (dual-row fp8 ISA rule)
    W2H = nc.dram_tensor("W2H", [128, 2, 272], f8, kind="ExternalInput")
    # CB packs lin consts ([:, 0:4, :]) + b2 halves ([:, 4, 0:2])
    CB = nc.dram_tensor("CB", [128, 5, 3], f32, kind="ExternalInput")
    Y = nc.dram_tensor("Y", [128, NCHUNK, 4, 3], f32, kind="ExternalOutput")

    with tile.TileContext(nc) as tc:
        with (
            tc.tile_pool(name="wpool", bufs=1) as wpool,
            tc.tile_pool(name="xpool", bufs=2) as xpool,
            tc.tile_pool(name="h1pool", bufs=2) as h1pool,
            tc.tile_pool(name="h2pool", bufs=3) as h2pool,
            tc.tile_pool(name="spool", bufs=2) as spool,
            tc.tile_pool(name="bpool", bufs=3) as bpool,
            tc.tile_pool(name="psu", bufs=7, space="PSUM") as psu,
            tc.tile_pool(name="pshead", bufs=1, space="PSUM") as pshead,
        ):
            w1t = wpool.tile([40, 256], f8)
            nc.sync.dma_start(w1t[:], W1T[:])
            w2h = wpool.tile([128, 2, 272], f8)
            nc.sync.dma_start(w2h[:], W2H[:])
            cb = wpool.tile([128, 5, 3], f32)
            nc.sync.dma_start(cb[:], CB[:])

            rep_ctx = tc.For_i(0, repeat) if repeat > 1 else nullcontext()
            with rep_ctx:
                _kernel_body(nc, tc, locals())

    nc.finalize()
    return nc


def _kernel_body(nc, tc, env):
    import os
    from concourse import bass

    ABL = set(os.environ.get("BASS_ABLATE", "").split(","))
    FILLER = int(os.environ.get("BASS_PE_FILLER", "0"))
    H1SBUFS = int(os.environ.get("BASS_H1SBUFS", "2"))

    mybir = bass.mybir
    f32 = mybir.dt.float32
    f8 = mybir.dt.float8e4
    DR = mybir.MatmulPerfMode.DoubleRow
    Relu = mybir.ActivationFunctionType.Relu
    add = mybir.AluOpType.add
    mult = mybir.AluOpType.mult
    amax = mybir.AluOpType.max
    PTT = nc.gpsimd.tensor_tensor
    XT, XRB, Y = env["XT"], env["XRB"], env["Y"]
    w1t, w2h, cb = env["w1t"], env["w2h"], env["cb"]
    xpool, h1pool, h2pool, spool, bpool = (
        env["xpool"], env["h1pool"], env["h2pool"], env["spool"],
        env["bpool"])
    psu, pshead = env["psu"], env["pshead"]

    h1s_static = None
    if "l2static" in ABL:
        h1s_static = h1pool.tile([128, 2, CHUNK], f8, name="h1stat", bufs=1)
        nc.gpsimd.memset(h1s_static[:], 0.25)

    xrb_t = [None] * NBATCH
    xt_t = [None] * NBATCH
    hp_t = [None] * NBATCH
    h1p_t = [None] * NCHUNK
    h1s_t = [None] * NCHUNK
    h2p_t = [None] * NCHUNK     # (slotA, slotB) PSUM ring tiles
    h2s_t = [None] * NCHUNK

    def stage_in(bi):
        base = bi * CPB * CHUNK
        xt = xpool.tile([40, CPB * CHUNK], f8, name="xt", bufs=2)
        if bi == 0:
            # split so chunk 0/1 land early and shorten the pipeline fill
            for lo, hi in ((0, CHUNK), (CHUNK, 2 * CHUNK),
                           (2 * CHUNK, CPB * CHUNK)):
                nc.sync.dma_start(xt[:, lo:hi],
                                  XT[:, base + lo:base + hi])
        else:
            nc.sync.dma_start(
                xt[:], XT[:, base:base + CPB * CHUNK])
        xt_t[bi] = xt
        xrb = bpool.tile([128, CPB, 4, 9], f32, name="xrb", bufs=3)
        nc.sync.dma_start(xrb[:], XRB[:, bi * CPB:(bi + 1) * CPB, :, :])
        xrb_t[bi] = xrb

    def epilogue(bi, off, n, suf):
        # y = ((p*bweps + ebias) * (lin + lb')) + (xr + bb)
        #   xrb cols: 0:3 bw*eps/SP^2, 3:6 xr+bb, 6:9 ebias/SP
        # hp copied PSUM->SBUF once (DVE); arithmetic on Pool (SBUF-only)
        hp = hp_t[bi]
        xrb = xrb_t[bi]
        hs = slice(off, off + n)
        hps = bpool.tile([128, n, 4, 6], f32, name="hps" + suf, bufs=2)
        hp_src = hp[:, hs, :, :] if hp is not None else xrb[:, hs, :, 0:6]
        nc.vector.tensor_scalar(hps[:], hp_src, 0.0, None,
                                op0=add)
        cb_lin = cb[:, None, 0:4, :].to_broadcast([128, n, 4, 3])
        linp = spool.tile([128, n, 4, 3], f32, name="linp" + suf, bufs=2)
        e = spool.tile([128, n, 4, 3], f32, name="e" + suf, bufs=2)
        ystg = bpool.tile([128, n, 4, 3], f32, name="ystg" + suf, bufs=2)
        PTT(out=linp[:], in0=hps[:, :, :, 0:3], in1=cb_lin, op=add)
        PTT(out=e[:], in0=hps[:, :, :, 3:6], in1=xrb[:, hs, :, 0:3],
            op=mult)
        PTT(out=e[:], in0=e[:], in1=xrb[:, hs, :, 6:9], op=add)
        PTT(out=e[:], in0=e[:], in1=linp[:], op=mult)
        PTT(out=ystg[:], in0=e[:], in1=xrb[:, hs, :, 3:6], op=add)
        nc.sync.dma_start(
            Y[:, bi * CPB + off:bi * CPB + off + n, :, :], ystg[:])

    # Software pipeline over chunks; at iteration ci every instruction's
    # inputs were produced in earlier iterations, so no engine waits on
    # another mid-period:
    #   PE : heads(ci-4), L1(ci), L2(ci-2)        (L2b emitted last)
    #   ACT: h2 castA(ci-3), castB(ci-3)
    #   DVE: h1 cast(ci-1) (+ hp copy at batch ends)
    #   Pool: epilogue arithmetic (SBUF only)
    # h2 PSUM ring-3: slot(L2a(c)) is freed by castA(c) early in iter c+3
    # and reused by L2b(c+1) late in the same iteration.
    stage_in(0)
    for ci in range(NCHUNK + 4):
        ck = ci - 4
        if ck >= 0 and "noheads" not in ABL:
            bi, cbk = divmod(ck, CPB)
            if cbk == 0:
                hp_t[bi] = pshead.tile([128, CPB, 4, 6], f32, name="hp",
                                       bufs=1)
            hp = hp_t[bi]
            h2s = h2s_t[ck]
            h2s_t[ck] = None
            # heads: hp[:, cbk, s, 0:3] = lin_mm, hp[:, cbk, s, 3:6] = p_mm
            # non-DR accumulating pairs: DR at FD=6 disables FWL and the
            # LDWEIGHTS overhead dwarfs the matmul
            for s in range(4):
                seg = slice(s * 128, (s + 1) * 128)
                nc.tensor.matmul(hp[:, cbk, s, :], h2s[:, 0, seg],
                                 w2h[:, 0, 256:262], start=True, stop=False)
                nc.tensor.matmul(hp[:, cbk, s, :], h2s[:, 1, seg],
                                 w2h[:, 1, 256:262], start=False, stop=True)

        ch = ci - 3
        if 0 <= ch < NCHUNK and "nocast2" not in ABL:
            # h2 casts on ACT: relu(h2 + 16*b2) per half, fp8 out at x16
            h2pa, h2pb = h2p_t[ch]
            h2p_t[ch] = None
            h2s = h2pool.tile([128, 2, CHUNK], f8, name="h2s", bufs=3)
            nc.scalar.activation(h2s[:, 0, :], h2pa[:], Relu,
                                 bias=cb[:, 4, 0:1], scale=1.0)
            nc.scalar.activation(h2s[:, 1, :], h2pb[:], Relu,
                                 bias=cb[:, 4, 1:2], scale=1.0)
            h2s_t[ch] = h2s

        if ci < NCHUNK:
            bi, cbk = divmod(ci, CPB)
            if cbk == 0 and bi + 1 < NBATCH:
                stage_in(bi + 1)
            # L1: H1T = W1T.T @ XT, bias via augmented row
            # weights host-scaled x64 for fp8; descaled in the DVE cast
            h1pa = psu.tile([128, CHUNK], f32, name="ps", bufs=7)
            h1pb = psu.tile([128, CHUNK], f32, name="ps", bufs=7)
            xt = xt_t[bi]
            sl = slice(cbk * CHUNK, (cbk + 1) * CHUNK)
            nc.tensor.matmul(h1pa[:], w1t[:, 0:128], xt[:, sl],
                             start=True, stop=True)
            nc.tensor.matmul(h1pb[:], w1t[:, 128:256], xt[:, sl],
                             start=True, stop=True)
            h1p_t[ci] = (h1pa, h1pb)

        ck1 = ci - 1
        if 0 <= ck1 < NCHUNK and "nocast1" not in ABL:
            # h1 cast on DVE: relu(h1p/64), one instr per half bank
            h1pa, h1pb = h1p_t[ck1]
            h1p_t[ck1] = None
            h1s = h1pool.tile([128, 2, CHUNK], f8, name="h1s",
                              bufs=H1SBUFS)
            nc.vector.tensor_scalar(h1s[:, 0, :], h1pa[:], 1.0 / 64, 0.0,
                                    op0=mult, op1=amax)
            nc.vector.tensor_scalar(h1s[:, 1, :], h1pb[:], 1.0 / 64, 0.0,
                                    op0=mult, op1=amax)
            h1s_t[ck1] = h1s

        cj = ci - 2
        if 0 <= cj < NCHUNK and "nol2" not in ABL:
            # L2: H2T = W2T.T @ H1T, one DoubleRow matmul per half into
            # the PSUM ring (W2/B2 host-scaled x16)
            h1s = h1s_t[cj]
            h1s_t[cj] = None
            if h1s_static is not None:
                h1s = h1s_static
            h2pa = psu.tile([128, CHUNK], f32, name="ps", bufs=7)
            h2pb = psu.tile([128, CHUNK], f32, name="ps", bufs=7)
            # dead-store duplicates keep the PE duty cycle high so DVFS
            # holds max clock; the real L2a write lands last
            for _ in range(FILLER):
                nc.tensor.matmul(h2pa[:], w2h[:, :, 0:128], h1s[:],
                                 start=True, stop=True, perf_mode=DR)
            nc.tensor.matmul(h2pa[:], w2h[:, :, 0:128], h1s[:],
                             start=True, stop=True, perf_mode=DR)
            nc.tensor.matmul(h2pb[:], w2h[:, :, 128:256], h1s[:],
                             start=True, stop=True, perf_mode=DR)
            h2p_t[cj] = (h2pa, h2pb)

        # epilogue after a batch of heads completes; final batch split in
        # two half-batches to shorten the pipeline drain
        if ck >= 0 and "noepi" not in ABL:
            if ck == NCHUNK - 5:
                epilogue(NBATCH - 1, 0, CPB - 4, "q")
            elif ck == NCHUNK - 1:
                epilogue(NBATCH - 1, CPB - 4, 4, "q")
            elif ck % CPB == CPB - 1:
                epilogue(ck // CPB, 0, CPB, "")


def _get_nc():
    if "nc" not in _NC_CACHE:
        _NC_CACHE["nc"] = _build_nc()
    return _NC_CACHE["nc"]


def _prepare_inputs(inputs):
    X = np.asarray(inputs["X"], dtype=np.float32)
    W1 = np.asarray(inputs["W1"], dtype=np.float32)
    b1 = np.asarray(inputs["b1"], dtype=np.float32)
    W2 = np.asarray(inputs["W2"], dtype=np.float32)
    b2 = np.asarray(inputs["b2"], dtype=np.float32)
    W3 = np.asarray(inputs["W3"], dtype=np.float32)
    b3 = np.asarray(inputs["b3"], dtype=np.float32)

    lw, lb, bew, beb, bw, bb = {}, {}, {}, {}, {}, {}
    for t in ("incl", "ecc", "mm"):
        lw[t] = np.asarray(inputs[f"lin_w_{t}"], np.float32)[0]        # [128]
        lb[t] = float(np.asarray(inputs[f"lin_b_{t}"], np.float32)[0])
        bew[t] = np.asarray(inputs[f"bile_w_{t}"], np.float32)[0][:, 0]  # [128]
        beb[t] = float(np.asarray(inputs[f"bile_b_{t}"], np.float32)[0])
        bw[t] = float(np.asarray(inputs[f"bil_w_{t}"], np.float32)[0, 0, 0])
        bb[t] = float(np.asarray(inputs[f"bil_b_{t}"], np.float32)[0])
    TS = ("incl", "ecc", "mm")
    RIDX = {"incl": 7, "ecc": 9, "mm": 12}

    # ---- replicated weights (fp8 DoubleRow layouts) ----
    # scales: W1 x64 (descaled in h1 DVE cast), W2/B2 x16 (h2 lives at
    # 16x in fp8, max |h2|<15 assumed), HW2 x16 -> hp at 256x; the 1/256
    # descale is folded exactly (powers of 2) into CONSTS/XRB.
    F8 = ml_dtypes.float8_e4m3
    SC1, SC2, SCH = 64.0, 16.0, 16.0
    SP = SC2 * SCH                                                  # 256
    W1a = np.zeros((40, 256), np.float32)
    W1a[0:D] = W1.T * SC1
    W1a[D] = b1 * SC1
    W1T = np.ascontiguousarray(W1a).astype(F8)
    O6 = np.stack([lw[t] for t in TS] + [bew[t] for t in TS], axis=1)  # [128,6]
    HW2f = W3.T.astype(np.float32) @ O6                             # [256, 6]
    W2H = np.zeros((128, 2, 272), np.float32)
    # cols 0:256: W2.T [k, m] at [k % 128, k // 128, oh*128 + m], x16
    W2H[:, :, 0:256] = (
        W2.T.reshape(2, 128, 256).transpose(1, 0, 2)) * SC2
    W2H[:, :, 256:262] = (
        HW2f.reshape(2, 128, 6).transpose(1, 0, 2)) * SCH
    W2H = W2H.astype(F8)
    c3 = np.array(
        [lb[t] + float(b3 @ lw[t]) for t in TS],         # lb' (b3 folded)
        dtype=np.float32) * SP
    CB = np.empty((128, 5, 3), np.float32)
    CB[:, 0:4, :] = c3
    CB[:, 4, 0] = b2[0:128] * SC2
    CB[:, 4, 1] = b2[128:256] * SC2
    CB[:, 4, 2] = 0.0
    K = {t: float(b3 @ bew[t]) for t in TS}

    in_maps = []
    for c in range(NCORES):
        Xl = X[c * ROWS:(c + 1) * ROWS]                             # [32768, 38]
        XTf = np.zeros((40, ROWS), np.float32)
        XTf[0:D] = Xl.T
        XTf[D] = 1.0
        XTc = np.ascontiguousarray(XTf).astype(F8)
        eps = Xl[:, 0]
        E9 = np.empty((ROWS, 9), np.float32)
        for j, t in enumerate(TS):
            E9[:, j] = bw[t] * eps / (SP * SP)
            E9[:, 3 + j] = Xl[:, RIDX[t]] + bb[t]
            E9[:, 6 + j] = (bw[t] * beb[t] + (bw[t] * K[t]) * eps) / SP
        XRBc = np.ascontiguousarray(
            E9.reshape(NCHUNK, 4, 128, 9).transpose(2, 0, 1, 3))
        in_maps.append({
            "XT": XTc, "XRB": XRBc, "W1T": W1T, "W2H": W2H, "CB": CB,
        })
    return in_maps


def _gather_output(results):
    Y = np.empty((B, 3), np.float32)
    for c in range(NCORES):
        Ydev = np.asarray(results[c]["Y"], np.float32)   # [128, 64, 4, 3]
        Y[c * ROWS:(c + 1) * ROWS] = (
            Ydev.transpose(1, 2, 0, 3).reshape(ROWS, 3))
    return Y


def run(inputs, trace=False, **spmd_kwargs):
    from concourse import bass_utils

    nc = _get_nc()
    in_maps = _prepare_inputs(inputs)
    res = bass_utils.run_bass_kernel_spmd(
        nc, in_maps, list(range(NCORES)), trace=trace, **spmd_kwargs)
    return _gather_output(res.results), res


def kernel(**inputs):
    out, _ = run(inputs)
    return out



# revision 3
# speedup vs baseline: 1.2028x; 1.2028x over previous
"""TRN2 Bass kernel for nn_NNModelEx_63513976373928.

Math (per row x of X [B, 38]):
  h1  = relu(x @ W1.T + b1)                  [256]
  h2  = relu(h1 @ W2.T + b2)                 [256]
  out = h2 @ W3.T + b3                       [128]
  per target t in (incl, ecc, mm), ridx in (7, 9, 12):
    lin = out . lw_t + lb_t
    e   = (out . bew_t) * eps + beb_t        eps = x[0]
    y_t = bw_t * e * lin + bb_t + x[ridx]

Device strategy (pure data parallel, 8 cores x 32768 rows):
  - feature-on-partition layout: H1T/H2T [units, rows], rows chunked by 512
  - b1 folded into the L1 matmul via an augmented contraction row
  - L3 + heads folded: lin/p are dots of h2 with W3.T @ lw / W3.T @ bew
  - X pre-transposed and fp8-cast on host -> XT [39, 32768] per core
  - residual/eps/bias columns packed fp32 on host -> XRB [128, 64, 4, 9]

v2 engine/bank plan (cast-engine bound; GPSIMD has no PSUM port):
  - DVE: single h1 cast instr/chunk ([128,2,512] PSUM->SBUF fp8) + one
    heads-PSUM->SBUF copy per 8-chunk batch
  - ACT: the two h2 half casts (per-half b2 bias forces 2 instrs)
  - Pool (GPSIMD): all epilogue arithmetic, SBUF-only
  - PE: heads(c-4), L1(c), L2(c-2) per iteration; all deps >= 1
    iteration old so no engine waits mid-period
  - PSUM 8 banks: h1p [128,2,512]x2 (4) + h2 ring-3 of [128,512] (3)
    + hp heads [128,8,4,6]x1 (1)
"""

import sys

for _p in ("/opt/trn_rl_repo", "/opt/trn_rl_repo/concourse"):
    if _p not in sys.path:
        sys.path.insert(0, _p)

import numpy as np
import ml_dtypes

BF16 = ml_dtypes.bfloat16

NCORES = 8
B = 262144
D = 38
DA = 39                     # contraction with bias row appended
ROWS = B // NCORES          # 32768 rows per core
CHUNK = 512                 # rows per chunk
NCHUNK = ROWS // CHUNK      # 64
CPB = 16                    # chunks per staging batch
NBATCH = NCHUNK // CPB      # 8

_NC_CACHE = {}


def _build_nc(repeat=1):
    from concourse import bass, bacc, tile
    from contextlib import nullcontext

    mybir = bass.mybir
    f32 = mybir.dt.float32
    f8 = mybir.dt.float8e4

    nc = bacc.Bacc(None, target_bir_lowering=False, debug=False)

    XT = nc.dram_tensor("XT", [40, ROWS], f8, kind="ExternalInput")
    XRB = nc.dram_tensor("XRB", [128, NCHUNK, 4, 9], f32, kind="ExternalInput")
    W1T = nc.dram_tensor("W1T", [40, 256], f8, kind="ExternalInput")
    # W2H packs W2 (cols 0:256, two output halves) + head vectors (256:262);
    # padded to 272 so the dim-1 stride is 16B-aligned (dual-row fp8 ISA rule)
    W2H = nc.dram_tensor("W2H", [128, 2, 272], f8, kind="ExternalInput")
    # CB packs lin consts ([:, 0:4, :]) + b2 halves ([:, 4, 0:2])
    CB = nc.dram_tensor("CB", [128, 5, 3], f32, kind="ExternalInput")
    Y = nc.dram_tensor("Y", [128, NCHUNK, 4, 3], f32, kind="ExternalOutput")

    with tile.TileContext(nc) as tc:
        with (
            tc.tile_pool(name="wpool", bufs=1) as wpool,
            tc.tile_pool(name="xpool", bufs=2) as xpool,
            tc.tile_pool(name="h1pool", bufs=2) as h1pool,
            tc.tile_pool(name="h2pool", bufs=3) as h2pool,
            tc.tile_pool(name="spool", bufs=2) as spool,
            tc.tile_pool(name="bpool", bufs=3) as bpool,
            tc.tile_pool(name="psu", bufs=7, space="PSUM") as psu,
            tc.tile_pool(name="pshead", bufs=1, space="PSUM") as pshead,
        ):
            w1t = wpool.tile([40, 256], f8)
            nc.sync.dma_start(w1t[:], W1T[:])
            w2h = wpool.tile([128, 2, 272], f8)
            nc.sync.dma_start(w2h[:], W2H[:])
            cb = wpool.tile([128, 5, 3], f32)
            nc.sync.dma_start(cb[:], CB[:])

            rep_ctx = tc.For_i(0, repeat) if repeat > 1 else nullcontext()
            with rep_ctx:
                _kernel_body(nc, tc, locals())

    nc.finalize()
    return nc


def _kernel_body(nc, tc, env):
    import os
    from concourse import bass

    ABL = set(os.environ.get("BASS_ABLATE", "").split(","))
    FILLER = int(os.environ.get("BASS_PE_FILLER", "0"))
    H1SBUFS = int(os.environ.get("BASS_H1SBUFS", "2"))

    mybir = bass.mybir
    f32 = mybir.dt.float32
    f8 = mybir.dt.float8e4
    DR = mybir.MatmulPerfMode.DoubleRow
    Relu = mybir.ActivationFunctionType.Relu
    add = mybir.AluOpType.add
    mult = mybir.AluOpType.mult
    amax = mybir.AluOpType.max
    PTT = nc.gpsimd.tensor_tensor
    XT, XRB, Y = env["XT"], env["XRB"], env["Y"]
    w1t, w2h, cb = env["w1t"], env["w2h"], env["cb"]
    xpool, h1pool, h2pool, spool, bpool = (
        env["xpool"], env["h1pool"], env["h2pool"], env["spool"],
        env["bpool"])
    psu, pshead = env["psu"], env["pshead"]

    h1s_static = None
    if "l2static" in ABL:
        h1s_static = h1pool.tile([128, 2, CHUNK], f8, name="h1stat", bufs=1)
        nc.gpsimd.memset(h1s_static[:], 0.25)

    xrb_t = [None] * NBATCH
    xt_t = [None] * NBATCH
    hp_t = [None] * NBATCH
    h1p_t = [None] * NCHUNK
    h1s_t = [None] * NCHUNK
    h2p_t = [None] * NCHUNK     # (slotA, slotB) PSUM ring tiles
    h2s_t = [None] * NCHUNK

    def stage_in(bi):
        base = bi * CPB * CHUNK
        xt = xpool.tile([40, CPB * CHUNK], f8, name="xt", bufs=2)
        if bi == 0:
            # split so chunk 0/1 land early and shorten the pipeline fill
            for lo, hi in ((0, CHUNK), (CHUNK, 2 * CHUNK),
                           (2 * CHUNK, CPB * CHUNK)):
                nc.sync.dma_start(xt[:, lo:hi],
                                  XT[:, base + lo:base + hi])
        else:
            nc.sync.dma_start(
                xt[:], XT[:, base:base + CPB * CHUNK])
        xt_t[bi] = xt
        xrb = bpool.tile([128, CPB, 4, 9], f32, name="xrb", bufs=3)
        nc.sync.dma_start(xrb[:], XRB[:, bi * CPB:(bi + 1) * CPB, :, :])
        xrb_t[bi] = xrb

    def epilogue(bi, off, n, suf):
        # y = ((p*bweps + ebias) * (lin + lb')) + (xr + bb)
        #   xrb cols: 0:3 bw*eps/SP^2, 3:6 xr+bb, 6:9 ebias/SP
        # hp copied PSUM->SBUF once (DVE); arithmetic on Pool (SBUF-only)
        hp = hp_t[bi]
        xrb = xrb_t[bi]
        hs = slice(off, off + n)
        hps = bpool.tile([128, n, 4, 6], f32, name="hps" + suf, bufs=2)
        hp_src = hp[:, hs, :, :] if hp is not None else xrb[:, hs, :, 0:6]
        nc.vector.tensor_scalar(hps[:], hp_src, 0.0, None,
                                op0=add)
        cb_lin = cb[:, None, 0:4, :].to_broadcast([128, n, 4, 3])
        linp = spool.tile([128, n, 4, 3], f32, name="linp" + suf, bufs=2)
        e = spool.tile([128, n, 4, 3], f32, name="e" + suf, bufs=2)
        ystg = bpool.tile([128, n, 4, 3], f32, name="ystg" + suf, bufs=2)
        PTT(out=linp[:], in0=hps[:, :, :, 0:3], in1=cb_lin, op=add)
        PTT(out=e[:], in0=hps[:, :, :, 3:6], in1=xrb[:, hs, :, 0:3],
            op=mult)
        PTT(out=e[:], in0=e[:], in1=xrb[:, hs, :, 6:9], op=add)
        PTT(out=e[:], in0=e[:], in1=linp[:], op=mult)
        PTT(out=ystg[:], in0=e[:], in1=xrb[:, hs, :, 3:6], op=add)
        nc.sync.dma_start(
            Y[:, bi * CPB + off:bi * CPB + off + n, :, :], ystg[:])

    # Software pipeline over chunks; at iteration ci every instruction's
    # inputs were produced in earlier iterations, so no engine waits on
    # another mid-period:
    #   PE : heads(ci-4), L1(ci), L2(ci-2)        (L2b emitted last)
    #   ACT: h2 castA(ci-3), castB(ci-3)
    #   DVE: h1 cast(ci-1) (+ hp copy at batch ends)
    #   Pool: epilogue arithmetic (SBUF only)
    # h2 PSUM ring-3: slot(L2a(c)) is freed by castA(c) early in iter c+3
    # and reused by L2b(c+1) late in the same iteration.
    stage_in(0)
    for ci in range(NCHUNK + 4):
        ring = {}
        cj0 = ci - 2
        if 0 <= cj0 < NCHUNK and "nol2" not in ABL:
            ring["h2pa"] = psu.tile([128, CHUNK], f32, name="ps", bufs=7)
            ring["h2pb"] = psu.tile([128, CHUNK], f32, name="ps", bufs=7)
        if ci < NCHUNK:
            ring["h1pa"] = psu.tile([128, CHUNK], f32, name="ps", bufs=7)
            ring["h1pb"] = psu.tile([128, CHUNK], f32, name="ps", bufs=7)
        ck = ci - 4
        if ck >= 0 and "noheads" not in ABL:
            bi, cbk = divmod(ck, CPB)
            if cbk == 0:
                hp_t[bi] = pshead.tile([128, CPB, 4, 6], f32, name="hp",
                                       bufs=1)
            hp = hp_t[bi]
            h2s = h2s_t[ck]
            h2s_t[ck] = None
            # heads: hp[:, cbk, s, 0:3] = lin_mm, hp[:, cbk, s, 3:6] = p_mm
            # non-DR accumulating pairs: DR at FD=6 disables FWL and the
            # LDWEIGHTS overhead dwarfs the matmul
            for s in range(4):
                seg = slice(s * 128, (s + 1) * 128)
                nc.tensor.matmul(hp[:, cbk, s, :], h2s[:, 0, seg],
                                 w2h[:, 0, 256:262], start=True, stop=False)
                nc.tensor.matmul(hp[:, cbk, s, :], h2s[:, 1, seg],
                                 w2h[:, 1, 256:262], start=False, stop=True)

        ch = ci - 3
        if 0 <= ch < NCHUNK and "nocast2" not in ABL:
            # h2 casts on ACT: relu(h2 + 16*b2) per half, fp8 out at x16
            h2pa, h2pb = h2p_t[ch]
            h2p_t[ch] = None
            h2s = h2pool.tile([128, 2, CHUNK], f8, name="h2s", bufs=3)
            nc.scalar.activation(h2s[:, 0, :], h2pa[:], Relu,
                                 bias=cb[:, 4, 0:1], scale=1.0)
            nc.scalar.activation(h2s[:, 1, :], h2pb[:], Relu,
                                 bias=cb[:, 4, 1:2], scale=1.0)
            h2s_t[ch] = h2s

        if ci < NCHUNK:
            bi, cbk = divmod(ci, CPB)
            if cbk == 0 and bi + 1 < NBATCH:
                stage_in(bi + 1)
            # L1: H1T = W1T.T @ XT, bias via augmented row
            # weights host-scaled x64 for fp8; descaled in the DVE cast
            h1pa = ring["h1pa"]
            h1pb = ring["h1pb"]
            xt = xt_t[bi]
            sl = slice(cbk * CHUNK, (cbk + 1) * CHUNK)
            nc.tensor.matmul(h1pa[:], w1t[:, 0:128], xt[:, sl],
                             start=True, stop=True)
            nc.tensor.matmul(h1pb[:], w1t[:, 128:256], xt[:, sl],
                             start=True, stop=True)
            h1p_t[ci] = (h1pa, h1pb)

        ck1 = ci - 1
        if 0 <= ck1 < NCHUNK and "nocast1" not in ABL:
            # h1 cast on DVE: relu(h1p/64), one instr per half bank
            h1pa, h1pb = h1p_t[ck1]
            h1p_t[ck1] = None
            h1s = h1pool.tile([128, 2, CHUNK], f8, name="h1s",
                              bufs=H1SBUFS)
            nc.vector.tensor_scalar(h1s[:, 0, :], h1pa[:], 1.0 / 64, 0.0,
                                    op0=mult, op1=amax)
            nc.vector.tensor_scalar(h1s[:, 1, :], h1pb[:], 1.0 / 64, 0.0,
                                    op0=mult, op1=amax)
            h1s_t[ck1] = h1s

        cj = ci - 2
        if 0 <= cj < NCHUNK and "nol2" not in ABL:
            # L2: H2T = W2T.T @ H1T, one DoubleRow matmul per half into
            # the PSUM ring (W2/B2 host-scaled x16)
            h1s = h1s_t[cj]
            h1s_t[cj] = None
            if h1s_static is not None:
                h1s = h1s_static
            h2pa = ring["h2pa"]
            h2pb = ring["h2pb"]
            # dead-store duplicates keep the PE duty cycle high so DVFS
            # holds max clock; the real L2a write lands last
            for _ in range(FILLER):
                nc.tensor.matmul(h2pa[:], w2h[:, :, 0:128], h1s[:],
                                 start=True, stop=True, perf_mode=DR)
            nc.tensor.matmul(h2pa[:], w2h[:, :, 0:128], h1s[:],
                             start=True, stop=True, perf_mode=DR)
            nc.tensor.matmul(h2pb[:], w2h[:, :, 128:256], h1s[:],
                             start=True, stop=True, perf_mode=DR)
            h2p_t[cj] = (h2pa, h2pb)

        # epilogue after a batch of heads completes; final batch split in
        # two half-batches to shorten the pipeline drain
        if ck >= 0 and "noepi" not in ABL:
            if ck == NCHUNK - 5:
                epilogue(NBATCH - 1, 0, CPB - 4, "q")
            elif ck == NCHUNK - 1:
                epilogue(NBATCH - 1, CPB - 4, 4, "q")
            elif ck % CPB == CPB - 1:
                epilogue(ck // CPB, 0, CPB, "")


def _get_nc():
    if "nc" not in _NC_CACHE:
        _NC_CACHE["nc"] = _build_nc()
    return _NC_CACHE["nc"]


def _prepare_inputs(inputs):
    X = np.asarray(inputs["X"], dtype=np.float32)
    W1 = np.asarray(inputs["W1"], dtype=np.float32)
    b1 = np.asarray(inputs["b1"], dtype=np.float32)
    W2 = np.asarray(inputs["W2"], dtype=np.float32)
    b2 = np.asarray(inputs["b2"], dtype=np.float32)
    W3 = np.asarray(inputs["W3"], dtype=np.float32)
    b3 = np.asarray(inputs["b3"], dtype=np.float32)

    lw, lb, bew, beb, bw, bb = {}, {}, {}, {}, {}, {}
    for t in ("incl", "ecc", "mm"):
        lw[t] = np.asarray(inputs[f"lin_w_{t}"], np.float32)[0]        # [128]
        lb[t] = float(np.asarray(inputs[f"lin_b_{t}"], np.float32)[0])
        bew[t] = np.asarray(inputs[f"bile_w_{t}"], np.float32)[0][:, 0]  # [128]
        beb[t] = float(np.asarray(inputs[f"bile_b_{t}"], np.float32)[0])
        bw[t] = float(np.asarray(inputs[f"bil_w_{t}"], np.float32)[0, 0, 0])
        bb[t] = float(np.asarray(inputs[f"bil_b_{t}"], np.float32)[0])
    TS = ("incl", "ecc", "mm")
    RIDX = {"incl": 7, "ecc": 9, "mm": 12}

    # ---- replicated weights (fp8 DoubleRow layouts) ----
    # scales: W1 x64 (descaled in h1 DVE cast), W2/B2 x16 (h2 lives at
    # 16x in fp8, max |h2|<15 assumed), HW2 x16 -> hp at 256x; the 1/256
    # descale is folded exactly (powers of 2) into CONSTS/XRB.
    F8 = ml_dtypes.float8_e4m3
    SC1, SC2, SCH = 64.0, 16.0, 16.0
    SP = SC2 * SCH                                                  # 256
    W1a = np.zeros((40, 256), np.float32)
    W1a[0:D] = W1.T * SC1
    W1a[D] = b1 * SC1
    W1T = np.ascontiguousarray(W1a).astype(F8)
    O6 = np.stack([lw[t] for t in TS] + [bew[t] for t in TS], axis=1)  # [128,6]
    HW2f = W3.T.astype(np.float32) @ O6                             # [256, 6]
    W2H = np.zeros((128, 2, 272), np.float32)
    # cols 0:256: W2.T [k, m] at [k % 128, k // 128, oh*128 + m], x16
    W2H[:, :, 0:256] = (
        W2.T.reshape(2, 128, 256).transpose(1, 0, 2)) * SC2
    W2H[:, :, 256:262] = (
        HW2f.reshape(2, 128, 6).transpose(1, 0, 2)) * SCH
    W2H = W2H.astype(F8)
    c3 = np.array(
        [lb[t] + float(b3 @ lw[t]) for t in TS],         # lb' (b3 folded)
        dtype=np.float32) * SP
    CB = np.empty((128, 5, 3), np.float32)
    CB[:, 0:4, :] = c3
    CB[:, 4, 0] = b2[0:128] * SC2
    CB[:, 4, 1] = b2[128:256] * SC2
    CB[:, 4, 2] = 0.0
    K = {t: float(b3 @ bew[t]) for t in TS}

    in_maps = []
    for c in range(NCORES):
        Xl = X[c * ROWS:(c + 1) * ROWS]                             # [32768, 38]
        XTf = np.zeros((40, ROWS), np.float32)
        XTf[0:D] = Xl.T
        XTf[D] = 1.0
        XTc = np.ascontiguousarray(XTf).astype(F8)
        eps = Xl[:, 0]
        E9 = np.empty((ROWS, 9), np.float32)
        for j, t in enumerate(TS):
            E9[:, j] = bw[t] * eps / (SP * SP)
            E9[:, 3 + j] = Xl[:, RIDX[t]] + bb[t]
            E9[:, 6 + j] = (bw[t] * beb[t] + (bw[t] * K[t]) * eps) / SP
        XRBc = np.ascontiguousarray(
            E9.reshape(NCHUNK, 4, 128, 9).transpose(2, 0, 1, 3))
        in_maps.append({
            "XT": XTc, "XRB": XRBc, "W1T": W1T, "W2H": W2H, "CB": CB,
        })
    return in_maps


def _gather_output(results):
    Y = np.empty((B, 3), np.float32)
    for c in range(NCORES):
        Ydev = np.asarray(results[c]["Y"], np.float32)   # [128, 64, 4, 3]
        Y[c * ROWS:(c + 1) * ROWS] = (
            Ydev.transpose(1, 2, 0, 3).reshape(ROWS, 3))
    return Y


def run(inputs, trace=False, **spmd_kwargs):
    from concourse import bass_utils

    nc = _get_nc()
    in_maps = _prepare_inputs(inputs)
    res = bass_utils.run_bass_kernel_spmd(
        nc, in_maps, list(range(NCORES)), trace=trace, **spmd_kwargs)
    return _gather_output(res.results), res


def kernel(**inputs):
    out, _ = run(inputs)
    return out



# revision 4
# speedup vs baseline: 1.2073x; 1.0037x over previous
"""TRN2 Bass kernel for nn_NNModelEx_63513976373928.

Math (per row x of X [B, 38]):
  h1  = relu(x @ W1.T + b1)                  [256]
  h2  = relu(h1 @ W2.T + b2)                 [256]
  out = h2 @ W3.T + b3                       [128]
  per target t in (incl, ecc, mm), ridx in (7, 9, 12):
    lin = out . lw_t + lb_t
    e   = (out . bew_t) * eps + beb_t        eps = x[0]
    y_t = bw_t * e * lin + bb_t + x[ridx]

Device strategy (pure data parallel, 8 cores x 32768 rows):
  - feature-on-partition layout: H1T/H2T [units, rows], rows chunked by 512
  - b1 folded into the L1 matmul via an augmented contraction row
  - L3 + heads folded: lin/p are dots of h2 with W3.T @ lw / W3.T @ bew
  - X pre-transposed and fp8-cast on host -> XT [39, 32768] per core
  - residual/eps/bias columns packed fp32 on host -> XRB [128, 64, 4, 9]

v3 engine/bank plan (cross-engine handoff bound):
  - PSUM: one unified ring pool of 7 one-bank [128,512] f32 tiles,
    4 allocs/chunk in slot order h2pa,h2pb,h1pa,h1pb (1.75 chunks of
    WAR slack on every PE-write-after-cast-read edge) + 1 heads bank
  - DVE: two h1 half casts/chunk ([128,512] PSUM->SBUF fp8, ~658ns
    each) + one heads-PSUM->SBUF copy per 16-chunk batch
  - ACT: the two h2 half casts (per-half b2 bias forces 2 instrs)
  - Pool (GPSIMD): all epilogue arithmetic, SBUF-only (no PSUM port)
  - PE: heads(c-4) emitted FIRST (tiny MMs absorb handoff latency),
    then L1(c) non-DR fp8 (K=40 -> FWL beats DoubleRow), L2(c-2) DR
  - CPB=16: fewer batch boundaries (DMA/epilogue/hp per 16 chunks)
  - HW-measured per-instr: DR MM FD512 295ns, non-DR 213ns, heads-MM
    51ns, DVE cast[128,512] 658ns, ACT act 690ns, Pool PTT 435ns;
    engines overlap fully when independent, but each cross-engine
    sem edge costs ~100-200ns -> minimize instructions and edges.
    Buffer-depth increases consistently HURT (scheduler placement).
  - v2 -> v3: 191us -> ~153us (R-slope); harness baseline was 223us
"""

import sys

for _p in ("/opt/trn_rl_repo", "/opt/trn_rl_repo/concourse"):
    if _p not in sys.path:
        sys.path.insert(0, _p)

import numpy as np
import ml_dtypes

BF16 = ml_dtypes.bfloat16

NCORES = 8
B = 262144
D = 38
DA = 39                     # contraction with bias row appended
ROWS = B // NCORES          # 32768 rows per core
CHUNK = 512                 # rows per chunk
NCHUNK = ROWS // CHUNK      # 64
CPB = 16                    # chunks per staging batch
NBATCH = NCHUNK // CPB      # 8

_NC_CACHE = {}


def _build_nc(repeat=1):
    from concourse import bass, bacc, tile
    from contextlib import nullcontext

    mybir = bass.mybir
    f32 = mybir.dt.float32
    f8 = mybir.dt.float8e4

    nc = bacc.Bacc(None, target_bir_lowering=False, debug=False)

    XT = nc.dram_tensor("XT", [40, ROWS], f8, kind="ExternalInput")
    XRB = nc.dram_tensor("XRB", [128, NCHUNK, 4, 9], f32, kind="ExternalInput")
    W1T = nc.dram_tensor("W1T", [40, 256], f8, kind="ExternalInput")
    # W2H packs W2 (cols 0:256, two output halves) + head vectors (256:262);
    # padded to 272 so the dim-1 stride is 16B-aligned (dual-row fp8 ISA rule)
    W2H = nc.dram_tensor("W2H", [128, 2, 272], f8, kind="ExternalInput")
    # CB packs lin consts ([:, 0:4, :]) + b2 halves ([:, 4, 0:2])
    CB = nc.dram_tensor("CB", [128, 5, 3], f32, kind="ExternalInput")
    Y = nc.dram_tensor("Y", [128, NCHUNK, 4, 3], f32, kind="ExternalOutput")

    with tile.TileContext(nc) as tc:
        with (
            tc.tile_pool(name="wpool", bufs=1) as wpool,
            tc.tile_pool(name="xpool", bufs=2) as xpool,
            tc.tile_pool(name="h1pool", bufs=2) as h1pool,
            tc.tile_pool(name="h2pool", bufs=3) as h2pool,
            tc.tile_pool(name="spool", bufs=2) as spool,
            tc.tile_pool(name="bpool", bufs=3) as bpool,
            tc.tile_pool(name="psu", bufs=7, space="PSUM") as psu,
            tc.tile_pool(name="pshead", bufs=1, space="PSUM") as pshead,
        ):
            w1t = wpool.tile([40, 256], f8)
            nc.sync.dma_start(w1t[:], W1T[:])
            w2h = wpool.tile([128, 2, 272], f8)
            nc.sync.dma_start(w2h[:], W2H[:])
            cb = wpool.tile([128, 5, 3], f32)
            nc.sync.dma_start(cb[:], CB[:])

            rep_ctx = tc.For_i(0, repeat) if repeat > 1 else nullcontext()
            with rep_ctx:
                _kernel_body(nc, tc, locals())

    nc.finalize()
    return nc


def _kernel_body(nc, tc, env):
    import os
    from concourse import bass

    ABL = set(os.environ.get("BASS_ABLATE", "").split(","))
    FILLER = int(os.environ.get("BASS_PE_FILLER", "0"))
    H1SBUFS = int(os.environ.get("BASS_H1SBUFS", "2"))

    mybir = bass.mybir
    f32 = mybir.dt.float32
    f8 = mybir.dt.float8e4
    DR = mybir.MatmulPerfMode.DoubleRow
    Relu = mybir.ActivationFunctionType.Relu
    add = mybir.AluOpType.add
    mult = mybir.AluOpType.mult
    amax = mybir.AluOpType.max
    PTT = nc.gpsimd.tensor_tensor
    XT, XRB, Y = env["XT"], env["XRB"], env["Y"]
    w1t, w2h, cb = env["w1t"], env["w2h"], env["cb"]
    xpool, h1pool, h2pool, spool, bpool = (
        env["xpool"], env["h1pool"], env["h2pool"], env["spool"],
        env["bpool"])
    psu, pshead = env["psu"], env["pshead"]

    h1s_static = None
    if "l2static" in ABL:
        h1s_static = h1pool.tile([128, 2, CHUNK], f8, name="h1stat", bufs=1)
        nc.gpsimd.memset(h1s_static[:], 0.25)

    xrb_t = [None] * NBATCH
    xt_t = [None] * NBATCH
    hp_t = [None] * NBATCH
    h1p_t = [None] * NCHUNK
    h1s_t = [None] * NCHUNK
    h2p_t = [None] * NCHUNK     # (slotA, slotB) PSUM ring tiles
    h2s_t = [None] * NCHUNK

    def stage_in(bi):
        base = bi * CPB * CHUNK
        xt = xpool.tile([40, CPB * CHUNK], f8, name="xt", bufs=2)
        if bi == 0:
            # split so chunk 0/1 land early and shorten the pipeline fill
            for lo, hi in ((0, CHUNK), (CHUNK, 2 * CHUNK),
                           (2 * CHUNK, CPB * CHUNK)):
                nc.sync.dma_start(xt[:, lo:hi],
                                  XT[:, base + lo:base + hi])
        else:
            nc.sync.dma_start(
                xt[:], XT[:, base:base + CPB * CHUNK])
        xt_t[bi] = xt
        xrb = bpool.tile([128, CPB, 4, 9], f32, name="xrb", bufs=3)
        nc.sync.dma_start(xrb[:], XRB[:, bi * CPB:(bi + 1) * CPB, :, :])
        xrb_t[bi] = xrb

    def epilogue(bi, off, n, suf):
        # y = ((p*bweps + ebias) * (lin + lb')) + (xr + bb)
        #   xrb cols: 0:3 bw*eps/SP^2, 3:6 xr+bb, 6:9 ebias/SP
        # hp copied PSUM->SBUF once (DVE); arithmetic on Pool (SBUF-only)
        hp = hp_t[bi]
        xrb = xrb_t[bi]
        hs = slice(off, off + n)
        hps = bpool.tile([128, n, 4, 6], f32, name="hps" + suf, bufs=2)
        hp_src = hp[:, hs, :, :] if hp is not None else xrb[:, hs, :, 0:6]
        nc.vector.tensor_scalar(hps[:], hp_src, 0.0, None,
                                op0=add)
        cb_lin = cb[:, None, 0:4, :].to_broadcast([128, n, 4, 3])
        linp = spool.tile([128, n, 4, 3], f32, name="linp" + suf, bufs=2)
        e = spool.tile([128, n, 4, 3], f32, name="e" + suf, bufs=2)
        ystg = bpool.tile([128, n, 4, 3], f32, name="ystg" + suf, bufs=2)
        PTT(out=linp[:], in0=hps[:, :, :, 0:3], in1=cb_lin, op=add)
        PTT(out=e[:], in0=hps[:, :, :, 3:6], in1=xrb[:, hs, :, 0:3],
            op=mult)
        PTT(out=e[:], in0=e[:], in1=xrb[:, hs, :, 6:9], op=add)
        PTT(out=e[:], in0=e[:], in1=linp[:], op=mult)
        PTT(out=ystg[:], in0=e[:], in1=xrb[:, hs, :, 3:6], op=add)
        nc.sync.dma_start(
            Y[:, bi * CPB + off:bi * CPB + off + n, :, :], ystg[:])

    # Software pipeline over chunks; at iteration ci every instruction's
    # inputs were produced in earlier iterations, so no engine waits on
    # another mid-period:
    #   PE : heads(ci-4), L1(ci), L2(ci-2)        (L2b emitted last)
    #   ACT: h2 castA(ci-3), castB(ci-3)
    #   DVE: h1 cast(ci-1) (+ hp copy at batch ends)
    #   Pool: epilogue arithmetic (SBUF only)
    # h2 PSUM ring-3: slot(L2a(c)) is freed by castA(c) early in iter c+3
    # and reused by L2b(c+1) late in the same iteration.
    stage_in(0)
    for ci in range(NCHUNK + 4):
        ring = {}
        cj0 = ci - 2
        if 0 <= cj0 < NCHUNK and "nol2" not in ABL:
            ring["h2pa"] = psu.tile([128, CHUNK], f32, name="ps", bufs=7)
            ring["h2pb"] = psu.tile([128, CHUNK], f32, name="ps", bufs=7)
        if ci < NCHUNK:
            ring["h1pa"] = psu.tile([128, CHUNK], f32, name="ps", bufs=7)
            ring["h1pb"] = psu.tile([128, CHUNK], f32, name="ps", bufs=7)
        ck = ci - 4
        if ck >= 0 and "noheads" not in ABL:
            bi, cbk = divmod(ck, CPB)
            if cbk == 0:
                hp_t[bi] = pshead.tile([128, CPB, 4, 6], f32, name="hp",
                                       bufs=1)
            hp = hp_t[bi]
            h2s = h2s_t[ck]
            h2s_t[ck] = None
            # heads: hp[:, cbk, s, 0:3] = lin_mm, hp[:, cbk, s, 3:6] = p_mm
            # non-DR accumulating pairs: DR at FD=6 disables FWL and the
            # LDWEIGHTS overhead dwarfs the matmul
            for s in range(4):
                seg = slice(s * 128, (s + 1) * 128)
                nc.tensor.matmul(hp[:, cbk, s, :], h2s[:, 0, seg],
                                 w2h[:, 0, 256:262], start=True, stop=False)
                nc.tensor.matmul(hp[:, cbk, s, :], h2s[:, 1, seg],
                                 w2h[:, 1, 256:262], start=False, stop=True)

        ch = ci - 3
        if 0 <= ch < NCHUNK and "nocast2" not in ABL:
            # h2 casts on ACT: relu(h2 + 16*b2) per half, fp8 out at x16
            h2pa, h2pb = h2p_t[ch]
            h2p_t[ch] = None
            h2s = h2pool.tile([128, 2, CHUNK], f8, name="h2s", bufs=3)
            nc.scalar.activation(h2s[:, 0, :], h2pa[:], Relu,
                                 bias=cb[:, 4, 0:1], scale=1.0)
            nc.scalar.activation(h2s[:, 1, :], h2pb[:], Relu,
                                 bias=cb[:, 4, 1:2], scale=1.0)
            h2s_t[ch] = h2s

        if ci < NCHUNK:
            bi, cbk = divmod(ci, CPB)
            if cbk == 0 and bi + 1 < NBATCH:
                stage_in(bi + 1)
            # L1: H1T = W1T.T @ XT, bias via augmented row
            # weights host-scaled x64 for fp8; descaled in the DVE cast
            h1pa = ring["h1pa"]
            h1pb = ring["h1pb"]
            xt = xt_t[bi]
            sl = slice(cbk * CHUNK, (cbk + 1) * CHUNK)
            nc.tensor.matmul(h1pa[:], w1t[:, 0:128], xt[:, sl],
                             start=True, stop=True)
            nc.tensor.matmul(h1pb[:], w1t[:, 128:256], xt[:, sl],
                             start=True, stop=True)
            h1p_t[ci] = (h1pa, h1pb)

        ck1 = ci - 1
        if 0 <= ck1 < NCHUNK and "nocast1" not in ABL:
            # h1 cast on DVE: relu(h1p/64), one instr per half bank
            h1pa, h1pb = h1p_t[ck1]
            h1p_t[ck1] = None
            h1s = h1pool.tile([128, 2, CHUNK], f8, name="h1s",
                              bufs=H1SBUFS)
            nc.vector.tensor_scalar(h1s[:, 0, :], h1pa[:], 1.0 / 64, 0.0,
                                    op0=mult, op1=amax)
            nc.vector.tensor_scalar(h1s[:, 1, :], h1pb[:], 1.0 / 64, 0.0,
                                    op0=mult, op1=amax)
            h1s_t[ck1] = h1s

        cj = ci - 2
        if 0 <= cj < NCHUNK and "nol2" not in ABL:
            # L2: H2T = W2T.T @ H1T, one DoubleRow matmul per half into
            # the PSUM ring (W2/B2 host-scaled x16)
            h1s = h1s_t[cj]
            h1s_t[cj] = None
            if h1s_static is not None:
                h1s = h1s_static
            h2pa = ring["h2pa"]
            h2pb = ring["h2pb"]
            # dead-store duplicates keep the PE duty cycle high so DVFS
            # holds max clock; the real L2a write lands last
            for _ in range(FILLER):
                nc.tensor.matmul(h2pa[:], w2h[:, :, 0:128], h1s[:],
                                 start=True, stop=True, perf_mode=DR)
            nc.tensor.matmul(h2pa[:], w2h[:, :, 0:128], h1s[:],
                             start=True, stop=True, perf_mode=DR)
            nc.tensor.matmul(h2pb[:], w2h[:, :, 128:256], h1s[:],
                             start=True, stop=True, perf_mode=DR)
            h2p_t[cj] = (h2pa, h2pb)

        # epilogue after a batch of heads completes; final batch split in
        # two half-batches to shorten the pipeline drain
        if ck >= 0 and "noepi" not in ABL:
            if ck == NCHUNK - 5:
                epilogue(NBATCH - 1, 0, CPB - 4, "q")
            elif ck == NCHUNK - 1:
                epilogue(NBATCH - 1, CPB - 4, 4, "q")
            elif ck % CPB == CPB - 1:
                epilogue(ck // CPB, 0, CPB, "")


def _get_nc():
    if "nc" not in _NC_CACHE:
        _NC_CACHE["nc"] = _build_nc()
    return _NC_CACHE["nc"]


def _prepare_inputs(inputs):
    X = np.asarray(inputs["X"], dtype=np.float32)
    W1 = np.asarray(inputs["W1"], dtype=np.float32)
    b1 = np.asarray(inputs["b1"], dtype=np.float32)
    W2 = np.asarray(inputs["W2"], dtype=np.float32)
    b2 = np.asarray(inputs["b2"], dtype=np.float32)
    W3 = np.asarray(inputs["W3"], dtype=np.float32)
    b3 = np.asarray(inputs["b3"], dtype=np.float32)

    lw, lb, bew, beb, bw, bb = {}, {}, {}, {}, {}, {}
    for t in ("incl", "ecc", "mm"):
        lw[t] = np.asarray(inputs[f"lin_w_{t}"], np.float32)[0]        # [128]
        lb[t] = float(np.asarray(inputs[f"lin_b_{t}"], np.float32)[0])
        bew[t] = np.asarray(inputs[f"bile_w_{t}"], np.float32)[0][:, 0]  # [128]
        beb[t] = float(np.asarray(inputs[f"bile_b_{t}"], np.float32)[0])
        bw[t] = float(np.asarray(inputs[f"bil_w_{t}"], np.float32)[0, 0, 0])
        bb[t] = float(np.asarray(inputs[f"bil_b_{t}"], np.float32)[0])
    TS = ("incl", "ecc", "mm")
    RIDX = {"incl": 7, "ecc": 9, "mm": 12}

    # ---- replicated weights (fp8 DoubleRow layouts) ----
    # scales: W1 x64 (descaled in h1 DVE cast), W2/B2 x16 (h2 lives at
    # 16x in fp8, max |h2|<15 assumed), HW2 x16 -> hp at 256x; the 1/256
    # descale is folded exactly (powers of 2) into CONSTS/XRB.
    F8 = ml_dtypes.float8_e4m3
    SC1, SC2, SCH = 64.0, 16.0, 16.0
    SP = SC2 * SCH                                                  # 256
    W1a = np.zeros((40, 256), np.float32)
    W1a[0:D] = W1.T * SC1
    W1a[D] = b1 * SC1
    W1T = np.ascontiguousarray(W1a).astype(F8)
    O6 = np.stack([lw[t] for t in TS] + [bew[t] for t in TS], axis=1)  # [128,6]
    HW2f = W3.T.astype(np.float32) @ O6                             # [256, 6]
    W2H = np.zeros((128, 2, 272), np.float32)
    # cols 0:256: W2.T [k, m] at [k % 128, k // 128, oh*128 + m], x16
    W2H[:, :, 0:256] = (
        W2.T.reshape(2, 128, 256).transpose(1, 0, 2)) * SC2
    W2H[:, :, 256:262] = (
        HW2f.reshape(2, 128, 6).transpose(1, 0, 2)) * SCH
    W2H = W2H.astype(F8)
    c3 = np.array(
        [lb[t] + float(b3 @ lw[t]) for t in TS],         # lb' (b3 folded)
        dtype=np.float32) * SP
    CB = np.empty((128, 5, 3), np.float32)
    CB[:, 0:4, :] = c3
    CB[:, 4, 0] = b2[0:128] * SC2
    CB[:, 4, 1] = b2[128:256] * SC2
    CB[:, 4, 2] = 0.0
    K = {t: float(b3 @ bew[t]) for t in TS}

    in_maps = []
    for c in range(NCORES):
        Xl = X[c * ROWS:(c + 1) * ROWS]                             # [32768, 38]
        XTf = np.zeros((40, ROWS), np.float32)
        XTf[0:D] = Xl.T
        XTf[D] = 1.0
        XTc = np.ascontiguousarray(XTf).astype(F8)
        eps = Xl[:, 0]
        E9 = np.empty((ROWS, 9), np.float32)
        for j, t in enumerate(TS):
            E9[:, j] = bw[t] * eps / (SP * SP)
            E9[:, 3 + j] = Xl[:, RIDX[t]] + bb[t]
            E9[:, 6 + j] = (bw[t] * beb[t] + (bw[t] * K[t]) * eps) / SP
        XRBc = np.ascontiguousarray(
            E9.reshape(NCHUNK, 4, 128, 9).transpose(2, 0, 1, 3))
        in_maps.append({
            "XT": XTc, "XRB": XRBc, "W1T": W1T, "W2H": W2H, "CB": CB,
        })
    return in_maps


def _gather_output(results):
    Y = np.empty((B, 3), np.float32)
    for c in range(NCORES):
        Ydev = np.asarray(results[c]["Y"], np.float32)   # [128, 64, 4, 3]
        Y[c * ROWS:(c + 1) * ROWS] = (
            Ydev.transpose(1, 2, 0, 3).reshape(ROWS, 3))
    return Y


def run(inputs, trace=False, **spmd_kwargs):
    from concourse import bass_utils

    nc = _get_nc()
    in_maps = _prepare_inputs(inputs)
    res = bass_utils.run_bass_kernel_spmd(
        nc, in_maps, list(range(NCORES)), trace=trace, **spmd_kwargs)
    return _gather_output(res.results), res


def kernel(**inputs):
    out, _ = run(inputs)
    return out



# revision 5
# speedup vs baseline: 1.2122x; 1.0041x over previous
"""TRN2 Bass kernel for nn_NNModelEx_63513976373928.

Math (per row x of X [B, 38]):
  h1  = relu(x @ W1.T + b1)                  [256]
  h2  = relu(h1 @ W2.T + b2)                 [256]
  out = h2 @ W3.T + b3                       [128]
  per target t in (incl, ecc, mm), ridx in (7, 9, 12):
    lin = out . lw_t + lb_t
    e   = (out . bew_t) * eps + beb_t        eps = x[0]
    y_t = bw_t * e * lin + bb_t + x[ridx]

Device strategy (pure data parallel, 8 cores x 32768 rows):
  - feature-on-partition layout: H1T/H2T [units, rows], rows chunked by 512
  - b1 folded into the L1 matmul via an augmented contraction row
  - L3 + heads folded: lin/p are dots of h2 with W3.T @ lw / W3.T @ bew
  - X pre-transposed and fp8-cast on host -> XT [39, 32768] per core
  - residual/eps/bias columns packed fp32 on host -> XRB [128, 64, 4, 9]

v3 engine/bank plan (cross-engine handoff bound):
  - PSUM: one unified ring pool of 7 one-bank [128,512] f32 tiles,
    4 allocs/chunk in slot order h2pa,h2pb,h1pa,h1pb (1.75 chunks of
    WAR slack on every PE-write-after-cast-read edge) + 1 heads bank
  - DVE: two h1 half casts/chunk ([128,512] PSUM->SBUF fp8, ~658ns
    each) + one heads-PSUM->SBUF copy per 16-chunk batch
  - ACT: the two h2 half casts (per-half b2 bias forces 2 instrs)
  - Pool (GPSIMD): all epilogue arithmetic, SBUF-only (no PSUM port)
  - PE: heads(c-4) emitted FIRST (tiny MMs absorb handoff latency),
    then L1(c) non-DR fp8 (K=40 -> FWL beats DoubleRow), L2(c-2) DR
  - CPB=16; batch-0 chunk-0 input hoisted ahead of the w2h/cb weight
    DMAs so the first matmul starts ~4us earlier in a one-shot run
  - HW-measured per-instr: DR MM FD512 295ns, non-DR 213ns, heads-MM
    51ns, DVE cast[128,512] 658ns, ACT act 690ns, Pool PTT 435ns;
    engines overlap fully when independent, but each cross-engine
    sem edge costs ~100-200ns -> minimize instructions and edges.
    Buffer/lag/priority changes in EITHER direction measured worse.
  - v2 -> v3: 191us -> ~153us (R-slope, healthy device state)
"""

import sys

for _p in ("/opt/trn_rl_repo", "/opt/trn_rl_repo/concourse"):
    if _p not in sys.path:
        sys.path.insert(0, _p)

import numpy as np
import ml_dtypes

BF16 = ml_dtypes.bfloat16

NCORES = 8
B = 262144
D = 38
DA = 39                     # contraction with bias row appended
ROWS = B // NCORES          # 32768 rows per core
CHUNK = 512                 # rows per chunk
NCHUNK = ROWS // CHUNK      # 64
CPB = 16                    # chunks per staging batch
NBATCH = NCHUNK // CPB      # 8

_NC_CACHE = {}


def _build_nc(repeat=1):
    from concourse import bass, bacc, tile
    from contextlib import nullcontext

    mybir = bass.mybir
    f32 = mybir.dt.float32
    f8 = mybir.dt.float8e4

    nc = bacc.Bacc(None, target_bir_lowering=False, debug=False)

    XT = nc.dram_tensor("XT", [40, ROWS], f8, kind="ExternalInput")
    XRB = nc.dram_tensor("XRB", [128, NCHUNK, 4, 9], f32, kind="ExternalInput")
    W1T = nc.dram_tensor("W1T", [40, 256], f8, kind="ExternalInput")
    # W2H packs W2 (cols 0:256, two output halves) + head vectors (256:262);
    # padded to 272 so the dim-1 stride is 16B-aligned (dual-row fp8 ISA rule)
    W2H = nc.dram_tensor("W2H", [128, 2, 272], f8, kind="ExternalInput")
    # CB packs lin consts ([:, 0:4, :]) + b2 halves ([:, 4, 0:2])
    CB = nc.dram_tensor("CB", [128, 5, 3], f32, kind="ExternalInput")
    Y = nc.dram_tensor("Y", [128, NCHUNK, 4, 3], f32, kind="ExternalOutput")

    with tile.TileContext(nc) as tc:
        with (
            tc.tile_pool(name="wpool", bufs=1) as wpool,
            tc.tile_pool(name="xpool", bufs=2) as xpool,
            tc.tile_pool(name="h1pool", bufs=2) as h1pool,
            tc.tile_pool(name="h2pool", bufs=3) as h2pool,
            tc.tile_pool(name="spool", bufs=2) as spool,
            tc.tile_pool(name="bpool", bufs=3) as bpool,
            tc.tile_pool(name="psu", bufs=7, space="PSUM") as psu,
            tc.tile_pool(name="pshead", bufs=1, space="PSUM") as pshead,
        ):
            w1t = wpool.tile([40, 256], f8)
            nc.sync.dma_start(w1t[:], W1T[:])
            xt_pre = wpool.tile([40, CHUNK], f8, name="xtpre")
            nc.sync.dma_start(xt_pre[:], XT[:, 0:CHUNK])
            w2h = wpool.tile([128, 2, 272], f8)
            nc.sync.dma_start(w2h[:], W2H[:])
            cb = wpool.tile([128, 5, 3], f32)
            nc.sync.dma_start(cb[:], CB[:])

            rep_ctx = tc.For_i(0, repeat) if repeat > 1 else nullcontext()
            with rep_ctx:
                _kernel_body(nc, tc, locals())

    nc.finalize()
    return nc


def _kernel_body(nc, tc, env):
    import os
    from concourse import bass

    ABL = set(os.environ.get("BASS_ABLATE", "").split(","))
    FILLER = int(os.environ.get("BASS_PE_FILLER", "0"))
    H1SBUFS = int(os.environ.get("BASS_H1SBUFS", "2"))

    mybir = bass.mybir
    f32 = mybir.dt.float32
    f8 = mybir.dt.float8e4
    DR = mybir.MatmulPerfMode.DoubleRow
    Relu = mybir.ActivationFunctionType.Relu
    add = mybir.AluOpType.add
    mult = mybir.AluOpType.mult
    amax = mybir.AluOpType.max
    PTT = nc.gpsimd.tensor_tensor
    XT, XRB, Y = env["XT"], env["XRB"], env["Y"]
    w1t, w2h, cb = env["w1t"], env["w2h"], env["cb"]
    xt_pre = env["xt_pre"]
    xpool, h1pool, h2pool, spool, bpool = (
        env["xpool"], env["h1pool"], env["h2pool"], env["spool"],
        env["bpool"])
    psu, pshead = env["psu"], env["pshead"]

    h1s_static = None
    if "l2static" in ABL:
        h1s_static = h1pool.tile([128, 2, CHUNK], f8, name="h1stat", bufs=1)
        nc.gpsimd.memset(h1s_static[:], 0.25)

    xrb_t = [None] * NBATCH
    xt_t = [None] * NBATCH
    hp_t = [None] * NBATCH
    h1p_t = [None] * NCHUNK
    h1s_t = [None] * NCHUNK
    h2p_t = [None] * NCHUNK     # (slotA, slotB) PSUM ring tiles
    h2s_t = [None] * NCHUNK

    def stage_in(bi):
        base = bi * CPB * CHUNK
        xt = xpool.tile([40, CPB * CHUNK], f8, name="xt", bufs=2)
        if bi == 0:
            # chunk 0 lives in xt_pre (DMA'd before the big weight
            # tensors); chunk 1 split keeps the fill short
            for lo, hi in ((CHUNK, 2 * CHUNK),
                           (2 * CHUNK, CPB * CHUNK)):
                nc.sync.dma_start(xt[:, lo:hi],
                                  XT[:, base + lo:base + hi])
        else:
            nc.sync.dma_start(
                xt[:], XT[:, base:base + CPB * CHUNK])
        xt_t[bi] = xt
        xrb = bpool.tile([128, CPB, 4, 9], f32, name="xrb", bufs=3)
        nc.sync.dma_start(xrb[:], XRB[:, bi * CPB:(bi + 1) * CPB, :, :])
        xrb_t[bi] = xrb

    def epilogue(bi, off, n, suf):
        # y = ((p*bweps + ebias) * (lin + lb')) + (xr + bb)
        #   xrb cols: 0:3 bw*eps/SP^2, 3:6 xr+bb, 6:9 ebias/SP
        # hp copied PSUM->SBUF once (DVE); arithmetic on Pool (SBUF-only)
        hp = hp_t[bi]
        xrb = xrb_t[bi]
        hs = slice(off, off + n)
        hps = bpool.tile([128, n, 4, 6], f32, name="hps" + suf, bufs=2)
        hp_src = hp[:, hs, :, :] if hp is not None else xrb[:, hs, :, 0:6]
        nc.vector.tensor_scalar(hps[:], hp_src, 0.0, None,
                                op0=add)
        cb_lin = cb[:, None, 0:4, :].to_broadcast([128, n, 4, 3])
        linp = spool.tile([128, n, 4, 3], f32, name="linp" + suf, bufs=2)
        e = spool.tile([128, n, 4, 3], f32, name="e" + suf, bufs=2)
        ystg = bpool.tile([128, n, 4, 3], f32, name="ystg" + suf, bufs=2)
        PTT(out=linp[:], in0=hps[:, :, :, 0:3], in1=cb_lin, op=add)
        PTT(out=e[:], in0=hps[:, :, :, 3:6], in1=xrb[:, hs, :, 0:3],
            op=mult)
        PTT(out=e[:], in0=e[:], in1=xrb[:, hs, :, 6:9], op=add)
        PTT(out=e[:], in0=e[:], in1=linp[:], op=mult)
        PTT(out=ystg[:], in0=e[:], in1=xrb[:, hs, :, 3:6], op=add)
        nc.sync.dma_start(
            Y[:, bi * CPB + off:bi * CPB + off + n, :, :], ystg[:])

    # Software pipeline over chunks; at iteration ci every instruction's
    # inputs were produced in earlier iterations, so no engine waits on
    # another mid-period:
    #   PE : heads(ci-4), L1(ci), L2(ci-2)        (L2b emitted last)
    #   ACT: h2 castA(ci-3), castB(ci-3)
    #   DVE: h1 cast(ci-1) (+ hp copy at batch ends)
    #   Pool: epilogue arithmetic (SBUF only)
    # h2 PSUM ring-3: slot(L2a(c)) is freed by castA(c) early in iter c+3
    # and reused by L2b(c+1) late in the same iteration.
    stage_in(0)
    for ci in range(NCHUNK + 4):
        ring = {}
        cj0 = ci - 2
        if 0 <= cj0 < NCHUNK and "nol2" not in ABL:
            ring["h2pa"] = psu.tile([128, CHUNK], f32, name="ps", bufs=7)
            ring["h2pb"] = psu.tile([128, CHUNK], f32, name="ps", bufs=7)
        if ci < NCHUNK:
            ring["h1pa"] = psu.tile([128, CHUNK], f32, name="ps", bufs=7)
            ring["h1pb"] = psu.tile([128, CHUNK], f32, name="ps", bufs=7)
        ck = ci - 4
        if ck >= 0 and "noheads" not in ABL:
            bi, cbk = divmod(ck, CPB)
            if cbk == 0:
                hp_t[bi] = pshead.tile([128, CPB, 4, 6], f32, name="hp",
                                       bufs=1)
            hp = hp_t[bi]
            h2s = h2s_t[ck]
            h2s_t[ck] = None
            # heads: hp[:, cbk, s, 0:3] = lin_mm, hp[:, cbk, s, 3:6] = p_mm
            # non-DR accumulating pairs: DR at FD=6 disables FWL and the
            # LDWEIGHTS overhead dwarfs the matmul
            for s in range(4):
                seg = slice(s * 128, (s + 1) * 128)
                nc.tensor.matmul(hp[:, cbk, s, :], h2s[:, 0, seg],
                                 w2h[:, 0, 256:262], start=True, stop=False)
                nc.tensor.matmul(hp[:, cbk, s, :], h2s[:, 1, seg],
                                 w2h[:, 1, 256:262], start=False, stop=True)

        ch = ci - 3
        if 0 <= ch < NCHUNK and "nocast2" not in ABL:
            # h2 casts on ACT: relu(h2 + 16*b2) per half, fp8 out at x16
            h2pa, h2pb = h2p_t[ch]
            h2p_t[ch] = None
            h2s = h2pool.tile([128, 2, CHUNK], f8, name="h2s", bufs=3)
            nc.scalar.activation(h2s[:, 0, :], h2pa[:], Relu,
                                 bias=cb[:, 4, 0:1], scale=1.0)
            nc.scalar.activation(h2s[:, 1, :], h2pb[:], Relu,
                                 bias=cb[:, 4, 1:2], scale=1.0)
            h2s_t[ch] = h2s

        if ci < NCHUNK:
            bi, cbk = divmod(ci, CPB)
            if cbk == 0 and bi + 1 < NBATCH:
                stage_in(bi + 1)
            # L1: H1T = W1T.T @ XT, bias via augmented row
            # weights host-scaled x64 for fp8; descaled in the DVE cast
            h1pa = ring["h1pa"]
            h1pb = ring["h1pb"]
            xt = xt_t[bi]
            sl = slice(cbk * CHUNK, (cbk + 1) * CHUNK)
            xsrc = xt_pre[:, 0:CHUNK] if ci == 0 else xt[:, sl]
            nc.tensor.matmul(h1pa[:], w1t[:, 0:128], xsrc,
                             start=True, stop=True)
            nc.tensor.matmul(h1pb[:], w1t[:, 128:256], xsrc,
                             start=True, stop=True)
            h1p_t[ci] = (h1pa, h1pb)

        ck1 = ci - 1
        if 0 <= ck1 < NCHUNK and "nocast1" not in ABL:
            # h1 cast on DVE: relu(h1p/64), one instr per half bank
            h1pa, h1pb = h1p_t[ck1]
            h1p_t[ck1] = None
            h1s = h1pool.tile([128, 2, CHUNK], f8, name="h1s",
                              bufs=H1SBUFS)
            nc.vector.tensor_scalar(h1s[:, 0, :], h1pa[:], 1.0 / 64, 0.0,
                                    op0=mult, op1=amax)
            nc.vector.tensor_scalar(h1s[:, 1, :], h1pb[:], 1.0 / 64, 0.0,
                                    op0=mult, op1=amax)
            h1s_t[ck1] = h1s

        cj = ci - 2
        if 0 <= cj < NCHUNK and "nol2" not in ABL:
            # L2: H2T = W2T.T @ H1T, one DoubleRow matmul per half into
            # the PSUM ring (W2/B2 host-scaled x16)
            h1s = h1s_t[cj]
            h1s_t[cj] = None
            if h1s_static is not None:
                h1s = h1s_static
            h2pa = ring["h2pa"]
            h2pb = ring["h2pb"]
            # dead-store duplicates keep the PE duty cycle high so DVFS
            # holds max clock; the real L2a write lands last
            for _ in range(FILLER):
                nc.tensor.matmul(h2pa[:], w2h[:, :, 0:128], h1s[:],
                                 start=True, stop=True, perf_mode=DR)
            nc.tensor.matmul(h2pa[:], w2h[:, :, 0:128], h1s[:],
                             start=True, stop=True, perf_mode=DR)
            nc.tensor.matmul(h2pb[:], w2h[:, :, 128:256], h1s[:],
                             start=True, stop=True, perf_mode=DR)
            h2p_t[cj] = (h2pa, h2pb)

        # epilogue after a batch of heads completes; final batch split in
        # two half-batches to shorten the pipeline drain
        if ck >= 0 and "noepi" not in ABL:
            if ck == NCHUNK - 5:
                epilogue(NBATCH - 1, 0, CPB - 4, "q")
            elif ck == NCHUNK - 1:
                epilogue(NBATCH - 1, CPB - 4, 4, "q")
            elif ck % CPB == CPB - 1:
                epilogue(ck // CPB, 0, CPB, "")


def _get_nc():
    if "nc" not in _NC_CACHE:
        _NC_CACHE["nc"] = _build_nc()
    return _NC_CACHE["nc"]


def _prepare_inputs(inputs):
    X = np.asarray(inputs["X"], dtype=np.float32)
    W1 = np.asarray(inputs["W1"], dtype=np.float32)
    b1 = np.asarray(inputs["b1"], dtype=np.float32)
    W2 = np.asarray(inputs["W2"], dtype=np.float32)
    b2 = np.asarray(inputs["b2"], dtype=np.float32)
    W3 = np.asarray(inputs["W3"], dtype=np.float32)
    b3 = np.asarray(inputs["b3"], dtype=np.float32)

    lw, lb, bew, beb, bw, bb = {}, {}, {}, {}, {}, {}
    for t in ("incl", "ecc", "mm"):
        lw[t] = np.asarray(inputs[f"lin_w_{t}"], np.float32)[0]        # [128]
        lb[t] = float(np.asarray(inputs[f"lin_b_{t}"], np.float32)[0])
        bew[t] = np.asarray(inputs[f"bile_w_{t}"], np.float32)[0][:, 0]  # [128]
        beb[t] = float(np.asarray(inputs[f"bile_b_{t}"], np.float32)[0])
        bw[t] = float(np.asarray(inputs[f"bil_w_{t}"], np.float32)[0, 0, 0])
        bb[t] = float(np.asarray(inputs[f"bil_b_{t}"], np.float32)[0])
    TS = ("incl", "ecc", "mm")
    RIDX = {"incl": 7, "ecc": 9, "mm": 12}

    # ---- replicated weights (fp8 DoubleRow layouts) ----
    # scales: W1 x64 (descaled in h1 DVE cast), W2/B2 x16 (h2 lives at
    # 16x in fp8, max |h2|<15 assumed), HW2 x16 -> hp at 256x; the 1/256
    # descale is folded exactly (powers of 2) into CONSTS/XRB.
    F8 = ml_dtypes.float8_e4m3
    SC1, SC2, SCH = 64.0, 16.0, 16.0
    SP = SC2 * SCH                                                  # 256
    W1a = np.zeros((40, 256), np.float32)
    W1a[0:D] = W1.T * SC1
    W1a[D] = b1 * SC1
    W1T = np.ascontiguousarray(W1a).astype(F8)
    O6 = np.stack([lw[t] for t in TS] + [bew[t] for t in TS], axis=1)  # [128,6]
    HW2f = W3.T.astype(np.float32) @ O6                             # [256, 6]
    W2H = np.zeros((128, 2, 272), np.float32)
    # cols 0:256: W2.T [k, m] at [k % 128, k // 128, oh*128 + m], x16
    W2H[:, :, 0:256] = (
        W2.T.reshape(2, 128, 256).transpose(1, 0, 2)) * SC2
    W2H[:, :, 256:262] = (
        HW2f.reshape(2, 128, 6).transpose(1, 0, 2)) * SCH
    W2H = W2H.astype(F8)
    c3 = np.array(
        [lb[t] + float(b3 @ lw[t]) for t in TS],         # lb' (b3 folded)
        dtype=np.float32) * SP
    CB = np.empty((128, 5, 3), np.float32)
    CB[:, 0:4, :] = c3
    CB[:, 4, 0] = b2[0:128] * SC2
    CB[:, 4, 1] = b2[128:256] * SC2
    CB[:, 4, 2] = 0.0
    K = {t: float(b3 @ bew[t]) for t in TS}

    in_maps = []
    for c in range(NCORES):
        Xl = X[c * ROWS:(c + 1) * ROWS]                             # [32768, 38]
        XTf = np.zeros((40, ROWS), np.float32)
        XTf[0:D] = Xl.T
        XTf[D] = 1.0
        XTc = np.ascontiguousarray(XTf).astype(F8)
        eps = Xl[:, 0]
        E9 = np.empty((ROWS, 9), np.float32)
        for j, t in enumerate(TS):
            E9[:, j] = bw[t] * eps / (SP * SP)
            E9[:, 3 + j] = Xl[:, RIDX[t]] + bb[t]
            E9[:, 6 + j] = (bw[t] * beb[t] + (bw[t] * K[t]) * eps) / SP
        XRBc = np.ascontiguousarray(
            E9.reshape(NCHUNK, 4, 128, 9).transpose(2, 0, 1, 3))
        in_maps.append({
            "XT": XTc, "XRB": XRBc, "W1T": W1T, "W2H": W2H, "CB": CB,
        })
    return in_maps


def _gather_output(results):
    Y = np.empty((B, 3), np.float32)
    for c in range(NCORES):
        Ydev = np.asarray(results[c]["Y"], np.float32)   # [128, 64, 4, 3]
        Y[c * ROWS:(c + 1) * ROWS] = (
            Ydev.transpose(1, 2, 0, 3).reshape(ROWS, 3))
    return Y


def run(inputs, trace=False, **spmd_kwargs):
    from concourse import bass_utils

    nc = _get_nc()
    in_maps = _prepare_inputs(inputs)
    res = bass_utils.run_bass_kernel_spmd(
        nc, in_maps, list(range(NCORES)), trace=trace, **spmd_kwargs)
    return _gather_output(res.results), res


def kernel(**inputs):
    out, _ = run(inputs)
    return out



# revision 6
# speedup vs baseline: 1.2389x; 1.0220x over previous
"""TRN2 Bass kernel for nn_NNModelEx_63513976373928.

Math (per row x of X [B, 38]):
  h1  = relu(x @ W1.T + b1)                  [256]
  h2  = relu(h1 @ W2.T + b2)                 [256]
  out = h2 @ W3.T + b3                       [128]
  per target t in (incl, ecc, mm), ridx in (7, 9, 12):
    lin = out . lw_t + lb_t
    e   = (out . bew_t) * eps + beb_t        eps = x[0]
    y_t = bw_t * e * lin + bb_t + x[ridx]

Device strategy (pure data parallel, 8 cores x 32768 rows):
  - feature-on-partition layout: H1T/H2T [units, rows], rows chunked by 512
  - b1 folded into the L1 matmul via an augmented contraction row
  - L3 + heads folded: lin/p are dots of h2 with W3.T @ lw / W3.T @ bew
  - X pre-transposed and fp8-cast on host -> XT [39, 32768] per core
  - residual/eps/bias columns packed fp32 on host -> XRB [128, 64, 4, 9]

v4 engine/bank plan (cross-engine handoff bound):
  - PSUM: h1p [128,2,512] f32 x2 bufs (4 banks) + h2 ring-3 of
    one-bank [128,512] tiles (3 banks) + 1 heads bank
  - DVE: ONE h1 cast instr/chunk ([128,2,512] PSUM->SBUF fp8, 1175ns
    — fewer instructions/sem edges beats two 658ns halves) + one
    heads-PSUM->SBUF copy per 16-chunk batch
  - ACT: the two h2 half casts (per-half b2 bias forces 2 instrs)
  - Pool (GPSIMD): all epilogue arithmetic, SBUF-only (no PSUM port)
  - PE: heads(c-4) emitted FIRST (tiny MMs absorb handoff latency),
    then L1(c) non-DR fp8 (K=40 -> FWL beats DoubleRow), L2(c-2) DR
  - CPB=16; batch-0 chunk-0 input hoisted ahead of the w2h/cb weight
    DMAs so the first matmul starts ~4us earlier in a one-shot run
  - HW-measured per-instr: DR MM FD512 295ns, non-DR 213ns, heads-MM
    51ns, DVE cast 1175ns, ACT act 690ns; engines overlap fully when
    independent; each cross-engine sem edge costs ~100-200ns ->
    minimize instructions and edges. Buffer/lag/priority changes in
    EITHER direction measured worse (sharp local optimum).
  - v2 191us -> v3 153us -> v4 152us good-state / 184 vs 187
    degraded-state (R-slope; device throttles on minute timescales)
"""

import sys

for _p in ("/opt/trn_rl_repo", "/opt/trn_rl_repo/concourse"):
    if _p not in sys.path:
        sys.path.insert(0, _p)

import numpy as np
import ml_dtypes

BF16 = ml_dtypes.bfloat16

NCORES = 8
B = 262144
D = 38
DA = 39                     # contraction with bias row appended
ROWS = B // NCORES          # 32768 rows per core
CHUNK = 512                 # rows per chunk
NCHUNK = ROWS // CHUNK      # 64
CPB = 16                    # chunks per staging batch
NBATCH = NCHUNK // CPB      # 8

_NC_CACHE = {}


def _build_nc(repeat=1):
    from concourse import bass, bacc, tile
    from contextlib import nullcontext

    mybir = bass.mybir
    f32 = mybir.dt.float32
    f8 = mybir.dt.float8e4

    nc = bacc.Bacc(None, target_bir_lowering=False, debug=False)

    XT = nc.dram_tensor("XT", [40, ROWS], f8, kind="ExternalInput")
    XRB = nc.dram_tensor("XRB", [128, NCHUNK, 4, 9], f32, kind="ExternalInput")
    W1T = nc.dram_tensor("W1T", [40, 256], f8, kind="ExternalInput")
    # W2H packs W2 (cols 0:256, two output halves) + head vectors (256:262);
    # padded to 272 so the dim-1 stride is 16B-aligned (dual-row fp8 ISA rule)
    W2H = nc.dram_tensor("W2H", [128, 2, 272], f8, kind="ExternalInput")
    # CB packs lin consts ([:, 0:4, :]) + b2 halves ([:, 4, 0:2])
    CB = nc.dram_tensor("CB", [128, 5, 3], f32, kind="ExternalInput")
    Y = nc.dram_tensor("Y", [128, NCHUNK, 4, 3], f32, kind="ExternalOutput")

    with tile.TileContext(nc) as tc:
        with (
            tc.tile_pool(name="wpool", bufs=1) as wpool,
            tc.tile_pool(name="xpool", bufs=2) as xpool,
            tc.tile_pool(name="h1pool", bufs=2) as h1pool,
            tc.tile_pool(name="h2pool", bufs=3) as h2pool,
            tc.tile_pool(name="spool", bufs=2) as spool,
            tc.tile_pool(name="bpool", bufs=3) as bpool,
            tc.tile_pool(name="ps1", bufs=2, space="PSUM") as ps1,
            tc.tile_pool(name="ps2", bufs=3, space="PSUM") as ps2,
            tc.tile_pool(name="pshead", bufs=1, space="PSUM") as pshead,
        ):
            w1t = wpool.tile([40, 256], f8)
            nc.sync.dma_start(w1t[:], W1T[:])
            xt_pre = wpool.tile([40, CHUNK], f8, name="xtpre")
            nc.sync.dma_start(xt_pre[:], XT[:, 0:CHUNK])
            w2h = wpool.tile([128, 2, 272], f8)
            nc.sync.dma_start(w2h[:], W2H[:])
            cb = wpool.tile([128, 5, 3], f32)
            nc.sync.dma_start(cb[:], CB[:])

            rep_ctx = tc.For_i(0, repeat) if repeat > 1 else nullcontext()
            with rep_ctx:
                _kernel_body(nc, tc, locals())

    nc.finalize()
    return nc


def _kernel_body(nc, tc, env):
    import os
    from concourse import bass

    ABL = set(os.environ.get("BASS_ABLATE", "").split(","))
    FILLER = int(os.environ.get("BASS_PE_FILLER", "0"))
    H1SBUFS = int(os.environ.get("BASS_H1SBUFS", "2"))

    mybir = bass.mybir
    f32 = mybir.dt.float32
    f8 = mybir.dt.float8e4
    DR = mybir.MatmulPerfMode.DoubleRow
    Relu = mybir.ActivationFunctionType.Relu
    add = mybir.AluOpType.add
    mult = mybir.AluOpType.mult
    amax = mybir.AluOpType.max
    PTT = nc.gpsimd.tensor_tensor
    XT, XRB, Y = env["XT"], env["XRB"], env["Y"]
    w1t, w2h, cb = env["w1t"], env["w2h"], env["cb"]
    xt_pre = env["xt_pre"]
    xpool, h1pool, h2pool, spool, bpool = (
        env["xpool"], env["h1pool"], env["h2pool"], env["spool"],
        env["bpool"])
    ps1, ps2, pshead = env["ps1"], env["ps2"], env["pshead"]

    h1s_static = None
    if "l2static" in ABL:
        h1s_static = h1pool.tile([128, 2, CHUNK], f8, name="h1stat", bufs=1)
        nc.gpsimd.memset(h1s_static[:], 0.25)

    xrb_t = [None] * NBATCH
    xt_t = [None] * NBATCH
    hp_t = [None] * NBATCH
    h1p_t = [None] * NCHUNK
    h1s_t = [None] * NCHUNK
    h2p_t = [None] * NCHUNK     # (slotA, slotB) PSUM ring tiles
    h2s_t = [None] * NCHUNK

    def stage_in(bi):
        base = bi * CPB * CHUNK
        xt = xpool.tile([40, CPB * CHUNK], f8, name="xt", bufs=2)
        if bi == 0:
            # chunk 0 lives in xt_pre (DMA'd before the big weight
            # tensors); chunk 1 split keeps the fill short
            for lo, hi in ((CHUNK, 2 * CHUNK),
                           (2 * CHUNK, CPB * CHUNK)):
                nc.sync.dma_start(xt[:, lo:hi],
                                  XT[:, base + lo:base + hi])
        else:
            nc.sync.dma_start(
                xt[:], XT[:, base:base + CPB * CHUNK])
        xt_t[bi] = xt
        xrb = bpool.tile([128, CPB, 4, 9], f32, name="xrb", bufs=3)
        nc.sync.dma_start(xrb[:], XRB[:, bi * CPB:(bi + 1) * CPB, :, :])
        xrb_t[bi] = xrb

    def epilogue(bi, off, n, suf):
        # y = ((p*bweps + ebias) * (lin + lb')) + (xr + bb)
        #   xrb cols: 0:3 bw*eps/SP^2, 3:6 xr+bb, 6:9 ebias/SP
        # hp copied PSUM->SBUF once (DVE); arithmetic on Pool (SBUF-only)
        hp = hp_t[bi]
        xrb = xrb_t[bi]
        hs = slice(off, off + n)
        hps = bpool.tile([128, n, 4, 6], f32, name="hps" + suf, bufs=2)
        hp_src = hp[:, hs, :, :] if hp is not None else xrb[:, hs, :, 0:6]
        nc.vector.tensor_scalar(hps[:], hp_src, 0.0, None,
                                op0=add)
        cb_lin = cb[:, None, 0:4, :].to_broadcast([128, n, 4, 3])
        linp = spool.tile([128, n, 4, 3], f32, name="linp" + suf, bufs=2)
        e = spool.tile([128, n, 4, 3], f32, name="e" + suf, bufs=2)
        ystg = bpool.tile([128, n, 4, 3], f32, name="ystg" + suf, bufs=2)
        PTT(out=linp[:], in0=hps[:, :, :, 0:3], in1=cb_lin, op=add)
        PTT(out=e[:], in0=hps[:, :, :, 3:6], in1=xrb[:, hs, :, 0:3],
            op=mult)
        PTT(out=e[:], in0=e[:], in1=xrb[:, hs, :, 6:9], op=add)
        PTT(out=e[:], in0=e[:], in1=linp[:], op=mult)
        PTT(out=ystg[:], in0=e[:], in1=xrb[:, hs, :, 3:6], op=add)
        nc.sync.dma_start(
            Y[:, bi * CPB + off:bi * CPB + off + n, :, :], ystg[:])

    # Software pipeline over chunks; at iteration ci every instruction's
    # inputs were produced in earlier iterations, so no engine waits on
    # another mid-period:
    #   PE : heads(ci-4), L1(ci), L2(ci-2)        (L2b emitted last)
    #   ACT: h2 castA(ci-3), castB(ci-3)
    #   DVE: h1 cast(ci-1) (+ hp copy at batch ends)
    #   Pool: epilogue arithmetic (SBUF only)
    # h2 PSUM ring-3: slot(L2a(c)) is freed by castA(c) early in iter c+3
    # and reused by L2b(c+1) late in the same iteration.
    stage_in(0)
    for ci in range(NCHUNK + 4):
        ring = {}
        cj0 = ci - 2
        if 0 <= cj0 < NCHUNK and "nol2" not in ABL:
            ring["h2pa"] = ps2.tile([128, CHUNK], f32, name="p2", bufs=3)
            ring["h2pb"] = ps2.tile([128, CHUNK], f32, name="p2", bufs=3)
        if ci < NCHUNK:
            ring["h1p"] = ps1.tile([128, 2, CHUNK], f32, name="p1", bufs=2)
        ck = ci - 4
        if ck >= 0 and "noheads" not in ABL:
            bi, cbk = divmod(ck, CPB)
            if cbk == 0:
                hp_t[bi] = pshead.tile([128, CPB, 4, 6], f32, name="hp",
                                       bufs=1)
            hp = hp_t[bi]
            h2s = h2s_t[ck]
            h2s_t[ck] = None
            # heads: hp[:, cbk, s, 0:3] = lin_mm, hp[:, cbk, s, 3:6] = p_mm
            # non-DR accumulating pairs: DR at FD=6 disables FWL and the
            # LDWEIGHTS overhead dwarfs the matmul
            for s in range(4):
                seg = slice(s * 128, (s + 1) * 128)
                nc.tensor.matmul(hp[:, cbk, s, :], h2s[:, 0, seg],
                                 w2h[:, 0, 256:262], start=True, stop=False)
                nc.tensor.matmul(hp[:, cbk, s, :], h2s[:, 1, seg],
                                 w2h[:, 1, 256:262], start=False, stop=True)

        ch = ci - 3
        if 0 <= ch < NCHUNK and "nocast2" not in ABL:
            # h2 casts on ACT: relu(h2 + 16*b2) per half, fp8 out at x16
            h2pa, h2pb = h2p_t[ch]
            h2p_t[ch] = None
            h2s = h2pool.tile([128, 2, CHUNK], f8, name="h2s", bufs=3)
            nc.scalar.activation(h2s[:, 0, :], h2pa[:], Relu,
                                 bias=cb[:, 4, 0:1], scale=1.0)
            nc.scalar.activation(h2s[:, 1, :], h2pb[:], Relu,
                                 bias=cb[:, 4, 1:2], scale=1.0)
            h2s_t[ch] = h2s

        if ci < NCHUNK:
            bi, cbk = divmod(ci, CPB)
            if cbk == 0 and bi + 1 < NBATCH:
                stage_in(bi + 1)
            # L1: H1T = W1T.T @ XT, bias via augmented row
            # weights host-scaled x64 for fp8; descaled in the DVE cast
            h1p = ring["h1p"]
            xt = xt_t[bi]
            sl = slice(cbk * CHUNK, (cbk + 1) * CHUNK)
            xsrc = xt_pre[:, 0:CHUNK] if ci == 0 else xt[:, sl]
            nc.tensor.matmul(h1p[:, 0, :], w1t[:, 0:128], xsrc,
                             start=True, stop=True)
            nc.tensor.matmul(h1p[:, 1, :], w1t[:, 128:256], xsrc,
                             start=True, stop=True)
            h1p_t[ci] = h1p

        ck1 = ci - 1
        if 0 <= ck1 < NCHUNK and "nocast1" not in ABL:
            # h1 cast on DVE: relu(h1p/64) in one [128,2,512] instr
            h1p = h1p_t[ck1]
            h1p_t[ck1] = None
            h1s = h1pool.tile([128, 2, CHUNK], f8, name="h1s",
                              bufs=H1SBUFS)
            nc.vector.tensor_scalar(h1s[:], h1p[:], 1.0 / 64, 0.0,
                                    op0=mult, op1=amax)
            h1s_t[ck1] = h1s

        cj = ci - 2
        if 0 <= cj < NCHUNK and "nol2" not in ABL:
            # L2: H2T = W2T.T @ H1T, one DoubleRow matmul per half into
            # the PSUM ring (W2/B2 host-scaled x16)
            h1s = h1s_t[cj]
            h1s_t[cj] = None
            if h1s_static is not None:
                h1s = h1s_static
            h2pa = ring["h2pa"]
            h2pb = ring["h2pb"]
            # dead-store duplicates keep the PE duty cycle high so DVFS
            # holds max clock; the real L2a write lands last
            for _ in range(FILLER):
                nc.tensor.matmul(h2pa[:], w2h[:, :, 0:128], h1s[:],
                                 start=True, stop=True, perf_mode=DR)
            nc.tensor.matmul(h2pa[:], w2h[:, :, 0:128], h1s[:],
                             start=True, stop=True, perf_mode=DR)
            nc.tensor.matmul(h2pb[:], w2h[:, :, 128:256], h1s[:],
                             start=True, stop=True, perf_mode=DR)
            h2p_t[cj] = (h2pa, h2pb)

        # epilogue after a batch of heads completes; final batch split in
        # two half-batches to shorten the pipeline drain
        if ck >= 0 and "noepi" not in ABL:
            if ck == NCHUNK - 5:
                epilogue(NBATCH - 1, 0, CPB - 4, "q")
            elif ck == NCHUNK - 1:
                epilogue(NBATCH - 1, CPB - 4, 4, "q")
            elif ck % CPB == CPB - 1:
                epilogue(ck // CPB, 0, CPB, "")


def _get_nc():
    if "nc" not in _NC_CACHE:
        _NC_CACHE["nc"] = _build_nc()
    return _NC_CACHE["nc"]


def _prepare_inputs(inputs):
    X = np.asarray(inputs["X"], dtype=np.float32)
    W1 = np.asarray(inputs["W1"], dtype=np.float32)
    b1 = np.asarray(inputs["b1"], dtype=np.float32)
    W2 = np.asarray(inputs["W2"], dtype=np.float32)
    b2 = np.asarray(inputs["b2"], dtype=np.float32)
    W3 = np.asarray(inputs["W3"], dtype=np.float32)
    b3 = np.asarray(inputs["b3"], dtype=np.float32)

    lw, lb, bew, beb, bw, bb = {}, {}, {}, {}, {}, {}
    for t in ("incl", "ecc", "mm"):
        lw[t] = np.asarray(inputs[f"lin_w_{t}"], np.float32)[0]        # [128]
        lb[t] = float(np.asarray(inputs[f"lin_b_{t}"], np.float32)[0])
        bew[t] = np.asarray(inputs[f"bile_w_{t}"], np.float32)[0][:, 0]  # [128]
        beb[t] = float(np.asarray(inputs[f"bile_b_{t}"], np.float32)[0])
        bw[t] = float(np.asarray(inputs[f"bil_w_{t}"], np.float32)[0, 0, 0])
        bb[t] = float(np.asarray(inputs[f"bil_b_{t}"], np.float32)[0])
    TS = ("incl", "ecc", "mm")
    RIDX = {"incl": 7, "ecc": 9, "mm": 12}

    # ---- replicated weights (fp8 DoubleRow layouts) ----
    # scales: W1 x64 (descaled in h1 DVE cast), W2/B2 x16 (h2 lives at
    # 16x in fp8, max |h2|<15 assumed), HW2 x16 -> hp at 256x; the 1/256
    # descale is folded exactly (powers of 2) into CONSTS/XRB.
    F8 = ml_dtypes.float8_e4m3
    SC1, SC2, SCH = 64.0, 16.0, 16.0
    SP = SC2 * SCH                                                  # 256
    W1a = np.zeros((40, 256), np.float32)
    W1a[0:D] = W1.T * SC1
    W1a[D] = b1 * SC1
    W1T = np.ascontiguousarray(W1a).astype(F8)
    O6 = np.stack([lw[t] for t in TS] + [bew[t] for t in TS], axis=1)  # [128,6]
    HW2f = W3.T.astype(np.float32) @ O6                             # [256, 6]
    W2H = np.zeros((128, 2, 272), np.float32)
    # cols 0:256: W2.T [k, m] at [k % 128, k // 128, oh*128 + m], x16
    W2H[:, :, 0:256] = (
        W2.T.reshape(2, 128, 256).transpose(1, 0, 2)) * SC2
    W2H[:, :, 256:262] = (
        HW2f.reshape(2, 128, 6).transpose(1, 0, 2)) * SCH
    W2H = W2H.astype(F8)
    c3 = np.array(
        [lb[t] + float(b3 @ lw[t]) for t in TS],         # lb' (b3 folded)
        dtype=np.float32) * SP
    CB = np.empty((128, 5, 3), np.float32)
    CB[:, 0:4, :] = c3
    CB[:, 4, 0] = b2[0:128] * SC2
    CB[:, 4, 1] = b2[128:256] * SC2
    CB[:, 4, 2] = 0.0
    K = {t: float(b3 @ bew[t]) for t in TS}

    in_maps = []
    for c in range(NCORES):
        Xl = X[c * ROWS:(c + 1) * ROWS]                             # [32768, 38]
        XTf = np.zeros((40, ROWS), np.float32)
        XTf[0:D] = Xl.T
        XTf[D] = 1.0
        XTc = np.ascontiguousarray(XTf).astype(F8)
        eps = Xl[:, 0]
        E9 = np.empty((ROWS, 9), np.float32)
        for j, t in enumerate(TS):
            E9[:, j] = bw[t] * eps / (SP * SP)
            E9[:, 3 + j] = Xl[:, RIDX[t]] + bb[t]
            E9[:, 6 + j] = (bw[t] * beb[t] + (bw[t] * K[t]) * eps) / SP
        XRBc = np.ascontiguousarray(
            E9.reshape(NCHUNK, 4, 128, 9).transpose(2, 0, 1, 3))
        in_maps.append({
            "XT": XTc, "XRB": XRBc, "W1T": W1T, "W2H": W2H, "CB": CB,
        })
    return in_maps


def _gather_output(results):
    Y = np.empty((B, 3), np.float32)
    for c in range(NCORES):
        Ydev = np.asarray(results[c]["Y"], np.float32)   # [128, 64, 4, 3]
        Y[c * ROWS:(c + 1) * ROWS] = (
            Ydev.transpose(1, 2, 0, 3).reshape(ROWS, 3))
    return Y


def run(inputs, trace=False, **spmd_kwargs):
    from concourse import bass_utils

    nc = _get_nc()
    in_maps = _prepare_inputs(inputs)
    res = bass_utils.run_bass_kernel_spmd(
        nc, in_maps, list(range(NCORES)), trace=trace, **spmd_kwargs)
    return _gather_output(res.results), res


def kernel(**inputs):
    out, _ = run(inputs)
    return out



# revision 7
# speedup vs baseline: 1.2412x; 1.0019x over previous
"""TRN2 Bass kernel for nn_NNModelEx_63513976373928.

Math (per row x of X [B, 38]):
  h1  = relu(x @ W1.T + b1)                  [256]
  h2  = relu(h1 @ W2.T + b2)                 [256]
  out = h2 @ W3.T + b3                       [128]
  per target t in (incl, ecc, mm), ridx in (7, 9, 12):
    lin = out . lw_t + lb_t
    e   = (out . bew_t) * eps + beb_t        eps = x[0]
    y_t = bw_t * e * lin + bb_t + x[ridx]

Device strategy (pure data parallel, 8 cores x 32768 rows):
  - feature-on-partition layout: H1T/H2T [units, rows], rows chunked by 512
  - b1 folded into the L1 matmul via an augmented contraction row
  - L3 + heads folded: lin/p are dots of h2 with W3.T @ lw / W3.T @ bew
  - X pre-transposed and fp8-cast on host -> XT [39, 32768] per core
  - residual/eps/bias columns packed fp32 on host -> XRB [128, 64, 4, 9]

v5 engine/bank plan (cross-engine handoff bound):
  - PSUM: h1p [128,2,512] f32 x2 bufs (4 banks) + h2 ring-3 of
    one-bank [128,512] tiles (3 banks) + 1 heads bank
  - ACT: ONE h1 cast instr/chunk (relu(h1p/64) [128,2,512], no bias
    needed since b1 rides L1's augmented contraction row)
  - DVE: the two h2 half casts (tensor_scalar add-b2-vec + max) + the
    heads-PSUM->SBUF copies per 16-chunk batch
  - Pool (GPSIMD): all epilogue arithmetic, SBUF-only (no PSUM port)
  - PE: heads(c-4) emitted FIRST (tiny MMs absorb handoff latency),
    then L1(c) non-DR fp8 (K=40 -> FWL beats DoubleRow), L2(c-2) DR
  - CPB=16; batch-0 chunk-0 input hoisted ahead of the w2h/cb weight
    DMAs so the first matmul starts ~4us earlier in a one-shot run
  - design law (HW-measured): engines overlap fully when independent;
    each cross-engine sem edge costs ~100-200ns; instruction/edge
    COUNT dominates instruction SIZE. Buffer/lag/priority changes in
    EITHER direction measured worse (sharp local optimum).
  - v2 191us -> v3 153 -> v4 152 -> v5 148us (R-slope, good device
    state; the device throttles ~20% on minute timescales)
"""

import sys

for _p in ("/opt/trn_rl_repo", "/opt/trn_rl_repo/concourse"):
    if _p not in sys.path:
        sys.path.insert(0, _p)

import numpy as np
import ml_dtypes

BF16 = ml_dtypes.bfloat16

NCORES = 8
B = 262144
D = 38
DA = 39                     # contraction with bias row appended
ROWS = B // NCORES          # 32768 rows per core
CHUNK = 512                 # rows per chunk
NCHUNK = ROWS // CHUNK      # 64
CPB = 16                    # chunks per staging batch
NBATCH = NCHUNK // CPB      # 8

_NC_CACHE = {}


def _build_nc(repeat=1):
    from concourse import bass, bacc, tile
    from contextlib import nullcontext

    mybir = bass.mybir
    f32 = mybir.dt.float32
    f8 = mybir.dt.float8e4

    nc = bacc.Bacc(None, target_bir_lowering=False, debug=False)

    XT = nc.dram_tensor("XT", [40, ROWS], f8, kind="ExternalInput")
    XRB = nc.dram_tensor("XRB", [128, NCHUNK, 4, 9], f32, kind="ExternalInput")
    W1T = nc.dram_tensor("W1T", [40, 256], f8, kind="ExternalInput")
    # W2H packs W2 (cols 0:256, two output halves) + head vectors (256:262);
    # padded to 272 so the dim-1 stride is 16B-aligned (dual-row fp8 ISA rule)
    W2H = nc.dram_tensor("W2H", [128, 2, 272], f8, kind="ExternalInput")
    # CB packs lin consts ([:, 0:4, :]) + b2 halves ([:, 4, 0:2])
    CB = nc.dram_tensor("CB", [128, 5, 3], f32, kind="ExternalInput")
    Y = nc.dram_tensor("Y", [128, NCHUNK, 4, 3], f32, kind="ExternalOutput")

    with tile.TileContext(nc) as tc:
        with (
            tc.tile_pool(name="wpool", bufs=1) as wpool,
            tc.tile_pool(name="xpool", bufs=2) as xpool,
            tc.tile_pool(name="h1pool", bufs=2) as h1pool,
            tc.tile_pool(name="h2pool", bufs=3) as h2pool,
            tc.tile_pool(name="spool", bufs=2) as spool,
            tc.tile_pool(name="bpool", bufs=3) as bpool,
            tc.tile_pool(name="ps1", bufs=2, space="PSUM") as ps1,
            tc.tile_pool(name="ps2", bufs=3, space="PSUM") as ps2,
            tc.tile_pool(name="pshead", bufs=1, space="PSUM") as pshead,
        ):
            w1t = wpool.tile([40, 256], f8)
            nc.sync.dma_start(w1t[:], W1T[:])
            xt_pre = wpool.tile([40, CHUNK], f8, name="xtpre")
            nc.sync.dma_start(xt_pre[:], XT[:, 0:CHUNK])
            w2h = wpool.tile([128, 2, 272], f8)
            nc.sync.dma_start(w2h[:], W2H[:])
            cb = wpool.tile([128, 5, 3], f32)
            nc.sync.dma_start(cb[:], CB[:])

            rep_ctx = tc.For_i(0, repeat) if repeat > 1 else nullcontext()
            with rep_ctx:
                _kernel_body(nc, tc, locals())

    nc.finalize()
    return nc


def _kernel_body(nc, tc, env):
    import os
    from concourse import bass

    ABL = set(os.environ.get("BASS_ABLATE", "").split(","))
    FILLER = int(os.environ.get("BASS_PE_FILLER", "0"))
    H1SBUFS = int(os.environ.get("BASS_H1SBUFS", "2"))

    mybir = bass.mybir
    f32 = mybir.dt.float32
    f8 = mybir.dt.float8e4
    DR = mybir.MatmulPerfMode.DoubleRow
    Relu = mybir.ActivationFunctionType.Relu
    add = mybir.AluOpType.add
    mult = mybir.AluOpType.mult
    amax = mybir.AluOpType.max
    PTT = nc.gpsimd.tensor_tensor
    XT, XRB, Y = env["XT"], env["XRB"], env["Y"]
    w1t, w2h, cb = env["w1t"], env["w2h"], env["cb"]
    xt_pre = env["xt_pre"]
    xpool, h1pool, h2pool, spool, bpool = (
        env["xpool"], env["h1pool"], env["h2pool"], env["spool"],
        env["bpool"])
    ps1, ps2, pshead = env["ps1"], env["ps2"], env["pshead"]

    h1s_static = None
    if "l2static" in ABL:
        h1s_static = h1pool.tile([128, 2, CHUNK], f8, name="h1stat", bufs=1)
        nc.gpsimd.memset(h1s_static[:], 0.25)

    xrb_t = [None] * NBATCH
    xt_t = [None] * NBATCH
    hp_t = [None] * NBATCH
    h1p_t = [None] * NCHUNK
    h1s_t = [None] * NCHUNK
    h2p_t = [None] * NCHUNK     # (slotA, slotB) PSUM ring tiles
    h2s_t = [None] * NCHUNK

    def stage_in(bi):
        base = bi * CPB * CHUNK
        xt = xpool.tile([40, CPB * CHUNK], f8, name="xt", bufs=2)
        if bi == 0:
            # chunk 0 lives in xt_pre (DMA'd before the big weight
            # tensors); chunk 1 split keeps the fill short
            for lo, hi in ((CHUNK, 2 * CHUNK),
                           (2 * CHUNK, CPB * CHUNK)):
                nc.sync.dma_start(xt[:, lo:hi],
                                  XT[:, base + lo:base + hi])
        else:
            nc.sync.dma_start(
                xt[:], XT[:, base:base + CPB * CHUNK])
        xt_t[bi] = xt
        xrb = bpool.tile([128, CPB, 4, 9], f32, name="xrb", bufs=3)
        nc.sync.dma_start(xrb[:], XRB[:, bi * CPB:(bi + 1) * CPB, :, :])
        xrb_t[bi] = xrb

    def epilogue(bi, off, n, suf):
        # y = ((p*bweps + ebias) * (lin + lb')) + (xr + bb)
        #   xrb cols: 0:3 bw*eps/SP^2, 3:6 xr+bb, 6:9 ebias/SP
        # hp copied PSUM->SBUF once (DVE); arithmetic on Pool (SBUF-only)
        hp = hp_t[bi]
        xrb = xrb_t[bi]
        hs = slice(off, off + n)
        hps = bpool.tile([128, n, 4, 6], f32, name="hps" + suf, bufs=2)
        hp_src = hp[:, hs, :, :] if hp is not None else xrb[:, hs, :, 0:6]
        nc.vector.tensor_scalar(hps[:], hp_src, 0.0, None,
                                op0=add)
        cb_lin = cb[:, None, 0:4, :].to_broadcast([128, n, 4, 3])
        linp = spool.tile([128, n, 4, 3], f32, name="linp" + suf, bufs=2)
        e = spool.tile([128, n, 4, 3], f32, name="e" + suf, bufs=2)
        ystg = bpool.tile([128, n, 4, 3], f32, name="ystg" + suf, bufs=2)
        PTT(out=linp[:], in0=hps[:, :, :, 0:3], in1=cb_lin, op=add)
        PTT(out=e[:], in0=hps[:, :, :, 3:6], in1=xrb[:, hs, :, 0:3],
            op=mult)
        PTT(out=e[:], in0=e[:], in1=xrb[:, hs, :, 6:9], op=add)
        PTT(out=e[:], in0=e[:], in1=linp[:], op=mult)
        PTT(out=ystg[:], in0=e[:], in1=xrb[:, hs, :, 3:6], op=add)
        nc.sync.dma_start(
            Y[:, bi * CPB + off:bi * CPB + off + n, :, :], ystg[:])

    # Software pipeline over chunks; at iteration ci every instruction's
    # inputs were produced in earlier iterations, so no engine waits on
    # another mid-period:
    #   PE : heads(ci-4), L1(ci), L2(ci-2)        (L2b emitted last)
    #   ACT: h2 castA(ci-3), castB(ci-3)
    #   DVE: h1 cast(ci-1) (+ hp copy at batch ends)
    #   Pool: epilogue arithmetic (SBUF only)
    # h2 PSUM ring-3: slot(L2a(c)) is freed by castA(c) early in iter c+3
    # and reused by L2b(c+1) late in the same iteration.
    stage_in(0)
    for ci in range(NCHUNK + 4):
        ring = {}
        cj0 = ci - 2
        if 0 <= cj0 < NCHUNK and "nol2" not in ABL:
            ring["h2pa"] = ps2.tile([128, CHUNK], f32, name="p2", bufs=3)
            ring["h2pb"] = ps2.tile([128, CHUNK], f32, name="p2", bufs=3)
        if ci < NCHUNK:
            ring["h1p"] = ps1.tile([128, 2, CHUNK], f32, name="p1", bufs=2)
        ck = ci - 4
        if ck >= 0 and "noheads" not in ABL:
            bi, cbk = divmod(ck, CPB)
            if cbk == 0:
                hp_t[bi] = pshead.tile([128, CPB, 4, 6], f32, name="hp",
                                       bufs=1)
            hp = hp_t[bi]
            h2s = h2s_t[ck]
            h2s_t[ck] = None
            # heads: hp[:, cbk, s, 0:3] = lin_mm, hp[:, cbk, s, 3:6] = p_mm
            # non-DR accumulating pairs: DR at FD=6 disables FWL and the
            # LDWEIGHTS overhead dwarfs the matmul
            for s in range(4):
                seg = slice(s * 128, (s + 1) * 128)
                nc.tensor.matmul(hp[:, cbk, s, :], h2s[:, 0, seg],
                                 w2h[:, 0, 256:262], start=True, stop=False)
                nc.tensor.matmul(hp[:, cbk, s, :], h2s[:, 1, seg],
                                 w2h[:, 1, 256:262], start=False, stop=True)

        ch = ci - 3
        if 0 <= ch < NCHUNK and "nocast2" not in ABL:
            # h2 casts on DVE: relu(h2 + 16*b2) per half, fp8 out at x16
            h2pa, h2pb = h2p_t[ch]
            h2p_t[ch] = None
            h2s = h2pool.tile([128, 2, CHUNK], f8, name="h2s", bufs=3)
            nc.vector.tensor_scalar(h2s[:, 0, :], h2pa[:], cb[:, 4, 0:1],
                                    0.0, op0=add, op1=amax)
            nc.vector.tensor_scalar(h2s[:, 1, :], h2pb[:], cb[:, 4, 1:2],
                                    0.0, op0=add, op1=amax)
            h2s_t[ch] = h2s

        if ci < NCHUNK:
            bi, cbk = divmod(ci, CPB)
            if cbk == 0 and bi + 1 < NBATCH:
                stage_in(bi + 1)
            # L1: H1T = W1T.T @ XT, bias via augmented row
            # weights host-scaled x64 for fp8; descaled in the DVE cast
            h1p = ring["h1p"]
            xt = xt_t[bi]
            sl = slice(cbk * CHUNK, (cbk + 1) * CHUNK)
            xsrc = xt_pre[:, 0:CHUNK] if ci == 0 else xt[:, sl]
            nc.tensor.matmul(h1p[:, 0, :], w1t[:, 0:128], xsrc,
                             start=True, stop=True)
            nc.tensor.matmul(h1p[:, 1, :], w1t[:, 128:256], xsrc,
                             start=True, stop=True)
            h1p_t[ci] = h1p

        ck1 = ci - 1
        if 0 <= ck1 < NCHUNK and "nocast1" not in ABL:
            # h1 cast on ACT: relu(h1p/64) in one [128,2,512] instr
            # (b1 folded via the augmented L1 row, so no bias needed)
            h1p = h1p_t[ck1]
            h1p_t[ck1] = None
            h1s = h1pool.tile([128, 2, CHUNK], f8, name="h1s",
                              bufs=H1SBUFS)
            nc.scalar.activation(h1s[:], h1p[:], Relu, scale=1.0 / 64)
            h1s_t[ck1] = h1s

        cj = ci - 2
        if 0 <= cj < NCHUNK and "nol2" not in ABL:
            # L2: H2T = W2T.T @ H1T, one DoubleRow matmul per half into
            # the PSUM ring (W2/B2 host-scaled x16)
            h1s = h1s_t[cj]
            h1s_t[cj] = None
            if h1s_static is not None:
                h1s = h1s_static
            h2pa = ring["h2pa"]
            h2pb = ring["h2pb"]
            # dead-store duplicates keep the PE duty cycle high so DVFS
            # holds max clock; the real L2a write lands last
            for _ in range(FILLER):
                nc.tensor.matmul(h2pa[:], w2h[:, :, 0:128], h1s[:],
                                 start=True, stop=True, perf_mode=DR)
            nc.tensor.matmul(h2pa[:], w2h[:, :, 0:128], h1s[:],
                             start=True, stop=True, perf_mode=DR)
            nc.tensor.matmul(h2pb[:], w2h[:, :, 128:256], h1s[:],
                             start=True, stop=True, perf_mode=DR)
            h2p_t[cj] = (h2pa, h2pb)

        # epilogue after a batch of heads completes; final batch split in
        # two half-batches to shorten the pipeline drain
        if ck >= 0 and "noepi" not in ABL:
            if ck == NCHUNK - 5:
                epilogue(NBATCH - 1, 0, CPB - 4, "q")
            elif ck == NCHUNK - 1:
                epilogue(NBATCH - 1, CPB - 4, 4, "q")
            elif ck % CPB == CPB - 1:
                epilogue(ck // CPB, 0, CPB, "")


def _get_nc():
    if "nc" not in _NC_CACHE:
        _NC_CACHE["nc"] = _build_nc()
    return _NC_CACHE["nc"]


def _prepare_inputs(inputs):
    X = np.asarray(inputs["X"], dtype=np.float32)
    W1 = np.asarray(inputs["W1"], dtype=np.float32)
    b1 = np.asarray(inputs["b1"], dtype=np.float32)
    W2 = np.asarray(inputs["W2"], dtype=np.float32)
    b2 = np.asarray(inputs["b2"], dtype=np.float32)
    W3 = np.asarray(inputs["W3"], dtype=np.float32)
    b3 = np.asarray(inputs["b3"], dtype=np.float32)

    lw, lb, bew, beb, bw, bb = {}, {}, {}, {}, {}, {}
    for t in ("incl", "ecc", "mm"):
        lw[t] = np.asarray(inputs[f"lin_w_{t}"], np.float32)[0]        # [128]
        lb[t] = float(np.asarray(inputs[f"lin_b_{t}"], np.float32)[0])
        bew[t] = np.asarray(inputs[f"bile_w_{t}"], np.float32)[0][:, 0]  # [128]
        beb[t] = float(np.asarray(inputs[f"bile_b_{t}"], np.float32)[0])
        bw[t] = float(np.asarray(inputs[f"bil_w_{t}"], np.float32)[0, 0, 0])
        bb[t] = float(np.asarray(inputs[f"bil_b_{t}"], np.float32)[0])
    TS = ("incl", "ecc", "mm")
    RIDX = {"incl": 7, "ecc": 9, "mm": 12}

    # ---- replicated weights (fp8 DoubleRow layouts) ----
    # scales: W1 x64 (descaled in h1 DVE cast), W2/B2 x16 (h2 lives at
    # 16x in fp8, max |h2|<15 assumed), HW2 x16 -> hp at 256x; the 1/256
    # descale is folded exactly (powers of 2) into CONSTS/XRB.
    F8 = ml_dtypes.float8_e4m3
    SC1, SC2, SCH = 64.0, 16.0, 16.0
    SP = SC2 * SCH                                                  # 256
    W1a = np.zeros((40, 256), np.float32)
    W1a[0:D] = W1.T * SC1
    W1a[D] = b1 * SC1
    W1T = np.ascontiguousarray(W1a).astype(F8)
    O6 = np.stack([lw[t] for t in TS] + [bew[t] for t in TS], axis=1)  # [128,6]
    HW2f = W3.T.astype(np.float32) @ O6                             # [256, 6]
    W2H = np.zeros((128, 2, 272), np.float32)
    # cols 0:256: W2.T [k, m] at [k % 128, k // 128, oh*128 + m], x16
    W2H[:, :, 0:256] = (
        W2.T.reshape(2, 128, 256).transpose(1, 0, 2)) * SC2
    W2H[:, :, 256:262] = (
        HW2f.reshape(2, 128, 6).transpose(1, 0, 2)) * SCH
    W2H = W2H.astype(F8)
    c3 = np.array(
        [lb[t] + float(b3 @ lw[t]) for t in TS],         # lb' (b3 folded)
        dtype=np.float32) * SP
    CB = np.empty((128, 5, 3), np.float32)
    CB[:, 0:4, :] = c3
    CB[:, 4, 0] = b2[0:128] * SC2
    CB[:, 4, 1] = b2[128:256] * SC2
    CB[:, 4, 2] = 0.0
    K = {t: float(b3 @ bew[t]) for t in TS}

    in_maps = []
    for c in range(NCORES):
        Xl = X[c * ROWS:(c + 1) * ROWS]                             # [32768, 38]
        XTf = np.zeros((40, ROWS), np.float32)
        XTf[0:D] = Xl.T
        XTf[D] = 1.0
        XTc = np.ascontiguousarray(XTf).astype(F8)
        eps = Xl[:, 0]
        E9 = np.empty((ROWS, 9), np.float32)
        for j, t in enumerate(TS):
            E9[:, j] = bw[t] * eps / (SP * SP)
            E9[:, 3 + j] = Xl[:, RIDX[t]] + bb[t]
            E9[:, 6 + j] = (bw[t] * beb[t] + (bw[t] * K[t]) * eps) / SP
        XRBc = np.ascontiguousarray(
            E9.reshape(NCHUNK, 4, 128, 9).transpose(2, 0, 1, 3))
        in_maps.append({
            "XT": XTc, "XRB": XRBc, "W1T": W1T, "W2H": W2H, "CB": CB,
        })
    return in_maps


def _gather_output(results):
    Y = np.empty((B, 3), np.float32)
    for c in range(NCORES):
        Ydev = np.asarray(results[c]["Y"], np.float32)   # [128, 64, 4, 3]
        Y[c * ROWS:(c + 1) * ROWS] = (
            Ydev.transpose(1, 2, 0, 3).reshape(ROWS, 3))
    return Y


def run(inputs, trace=False, **spmd_kwargs):
    from concourse import bass_utils

    nc = _get_nc()
    in_maps = _prepare_inputs(inputs)
    res = bass_utils.run_bass_kernel_spmd(
        nc, in_maps, list(range(NCORES)), trace=trace, **spmd_kwargs)
    return _gather_output(res.results), res


def kernel(**inputs):
    out, _ = run(inputs)
    return out



# revision 8
# speedup vs baseline: 1.2518x; 1.0085x over previous
"""TRN2 Bass kernel for nn_NNModelEx_63513976373928.

Math (per row x of X [B, 38]):
  h1  = relu(x @ W1.T + b1)                  [256]
  h2  = relu(h1 @ W2.T + b2)                 [256]
  out = h2 @ W3.T + b3                       [128]
  per target t in (incl, ecc, mm), ridx in (7, 9, 12):
    lin = out . lw_t + lb_t
    e   = (out . bew_t) * eps + beb_t        eps = x[0]
    y_t = bw_t * e * lin + bb_t + x[ridx]

Device strategy (pure data parallel, 8 cores x 32768 rows):
  - feature-on-partition layout: H1T/H2T [units, rows], rows chunked by 512
  - b1 folded into the L1 matmul via an augmented contraction row
  - L3 + heads folded: lin/p are dots of h2 with W3.T @ lw / W3.T @ bew
  - X pre-transposed and fp8-cast on host -> XT [39, 32768] per core
  - residual/eps/bias columns packed fp32 on host -> XRB [128, 64, 4, 9]

v6 engine/bank plan (cross-engine handoff bound):
  - PSUM: h1p [128,2,512] f32 x2 bufs (4 banks) + h2 ring-3 of
    one-bank [128,512] tiles (3 banks) + 1 heads bank
  - ACT: ONE h1 cast instr/chunk (relu(h1p/64) [128,2,512], no bias —
    b1 rides L1's augmented contraction row) + the per-batch heads
    PSUM->SBUF staging copy (scalar.copy; Copy shares Relu's table)
  - DVE: the two h2 half casts (tensor_scalar add-b2-vec + max)
  - Pool (GPSIMD): all epilogue arithmetic, SBUF-only (no PSUM port)
  - PE: heads(c-4) emitted FIRST (tiny MMs absorb handoff latency),
    then L1(c) non-DR fp8 (K=40 -> FWL beats DoubleRow), L2(c-2) DR
  - CPB=16; batch-0 chunk-0 input hoisted ahead of the w2h/cb weight
    DMAs so the first matmul starts ~4us earlier in a one-shot run
  - design law (HW-measured): engines overlap fully when independent;
    each cross-engine sem edge costs ~100-200ns; instruction/edge
    COUNT dominates instruction SIZE; balance the two PSUM-capable
    cast engines. Buffer/lag/priority changes measured worse.
  - v2 191us -> v3 153 -> v4 152 -> v5 148 -> v6 ~150.6/151 bracket
    (R-slope, good device state; device throttles on minute scales)
"""

import sys

for _p in ("/opt/trn_rl_repo", "/opt/trn_rl_repo/concourse"):
    if _p not in sys.path:
        sys.path.insert(0, _p)

import numpy as np
import ml_dtypes

BF16 = ml_dtypes.bfloat16

NCORES = 8
B = 262144
D = 38
DA = 39                     # contraction with bias row appended
ROWS = B // NCORES          # 32768 rows per core
CHUNK = 512                 # rows per chunk
NCHUNK = ROWS // CHUNK      # 64
CPB = 16                    # chunks per staging batch
NBATCH = NCHUNK // CPB      # 8

_NC_CACHE = {}


def _build_nc(repeat=1):
    from concourse import bass, bacc, tile
    from contextlib import nullcontext

    mybir = bass.mybir
    f32 = mybir.dt.float32
    f8 = mybir.dt.float8e4

    nc = bacc.Bacc(None, target_bir_lowering=False, debug=False)

    XT = nc.dram_tensor("XT", [40, ROWS], f8, kind="ExternalInput")
    XRB = nc.dram_tensor("XRB", [128, NCHUNK, 4, 9], f32, kind="ExternalInput")
    W1T = nc.dram_tensor("W1T", [40, 256], f8, kind="ExternalInput")
    # W2H packs W2 (cols 0:256, two output halves) + head vectors (256:262);
    # padded to 272 so the dim-1 stride is 16B-aligned (dual-row fp8 ISA rule)
    W2H = nc.dram_tensor("W2H", [128, 2, 272], f8, kind="ExternalInput")
    # CB packs lin consts ([:, 0:4, :]) + b2 halves ([:, 4, 0:2])
    CB = nc.dram_tensor("CB", [128, 5, 3], f32, kind="ExternalInput")
    Y = nc.dram_tensor("Y", [128, NCHUNK, 4, 3], f32, kind="ExternalOutput")

    with tile.TileContext(nc) as tc:
        with (
            tc.tile_pool(name="wpool", bufs=1) as wpool,
            tc.tile_pool(name="xpool", bufs=2) as xpool,
            tc.tile_pool(name="h1pool", bufs=2) as h1pool,
            tc.tile_pool(name="h2pool", bufs=3) as h2pool,
            tc.tile_pool(name="spool", bufs=2) as spool,
            tc.tile_pool(name="bpool", bufs=3) as bpool,
            tc.tile_pool(name="ps1", bufs=2, space="PSUM") as ps1,
            tc.tile_pool(name="ps2", bufs=3, space="PSUM") as ps2,
            tc.tile_pool(name="pshead", bufs=1, space="PSUM") as pshead,
        ):
            w1t = wpool.tile([40, 256], f8)
            nc.sync.dma_start(w1t[:], W1T[:])
            xt_pre = wpool.tile([40, CHUNK], f8, name="xtpre")
            nc.sync.dma_start(xt_pre[:], XT[:, 0:CHUNK])
            w2h = wpool.tile([128, 2, 272], f8)
            nc.sync.dma_start(w2h[:], W2H[:])
            cb = wpool.tile([128, 5, 3], f32)
            nc.sync.dma_start(cb[:], CB[:])

            rep_ctx = tc.For_i(0, repeat) if repeat > 1 else nullcontext()
            with rep_ctx:
                _kernel_body(nc, tc, locals())

    nc.finalize()
    return nc


def _kernel_body(nc, tc, env):
    import os
    from concourse import bass

    ABL = set(os.environ.get("BASS_ABLATE", "").split(","))
    FILLER = int(os.environ.get("BASS_PE_FILLER", "0"))
    H1SBUFS = int(os.environ.get("BASS_H1SBUFS", "2"))

    mybir = bass.mybir
    f32 = mybir.dt.float32
    f8 = mybir.dt.float8e4
    DR = mybir.MatmulPerfMode.DoubleRow
    Relu = mybir.ActivationFunctionType.Relu
    add = mybir.AluOpType.add
    mult = mybir.AluOpType.mult
    amax = mybir.AluOpType.max
    PTT = nc.gpsimd.tensor_tensor
    XT, XRB, Y = env["XT"], env["XRB"], env["Y"]
    w1t, w2h, cb = env["w1t"], env["w2h"], env["cb"]
    xt_pre = env["xt_pre"]
    xpool, h1pool, h2pool, spool, bpool = (
        env["xpool"], env["h1pool"], env["h2pool"], env["spool"],
        env["bpool"])
    ps1, ps2, pshead = env["ps1"], env["ps2"], env["pshead"]

    h1s_static = None
    if "l2static" in ABL:
        h1s_static = h1pool.tile([128, 2, CHUNK], f8, name="h1stat", bufs=1)
        nc.gpsimd.memset(h1s_static[:], 0.25)

    xrb_t = [None] * NBATCH
    xt_t = [None] * NBATCH
    hp_t = [None] * NBATCH
    h1p_t = [None] * NCHUNK
    h1s_t = [None] * NCHUNK
    h2p_t = [None] * NCHUNK     # (slotA, slotB) PSUM ring tiles
    h2s_t = [None] * NCHUNK

    def stage_in(bi):
        base = bi * CPB * CHUNK
        xt = xpool.tile([40, CPB * CHUNK], f8, name="xt", bufs=2)
        if bi == 0:
            # chunk 0 lives in xt_pre (DMA'd before the big weight
            # tensors); chunk 1 split keeps the fill short
            for lo, hi in ((CHUNK, 2 * CHUNK),
                           (2 * CHUNK, CPB * CHUNK)):
                nc.sync.dma_start(xt[:, lo:hi],
                                  XT[:, base + lo:base + hi])
        else:
            nc.sync.dma_start(
                xt[:], XT[:, base:base + CPB * CHUNK])
        xt_t[bi] = xt
        xrb = bpool.tile([128, CPB, 4, 9], f32, name="xrb", bufs=3)
        nc.sync.dma_start(xrb[:], XRB[:, bi * CPB:(bi + 1) * CPB, :, :])
        xrb_t[bi] = xrb

    def epilogue(bi, off, n, suf):
        # y = ((p*bweps + ebias) * (lin + lb')) + (xr + bb)
        #   xrb cols: 0:3 bw*eps/SP^2, 3:6 xr+bb, 6:9 ebias/SP
        # hp copied PSUM->SBUF once (DVE); arithmetic on Pool (SBUF-only)
        hp = hp_t[bi]
        xrb = xrb_t[bi]
        hs = slice(off, off + n)
        hps = bpool.tile([128, n, 4, 6], f32, name="hps" + suf, bufs=2)
        hp_src = hp[:, hs, :, :] if hp is not None else xrb[:, hs, :, 0:6]
        nc.scalar.copy(hps[:], hp_src)
        cb_lin = cb[:, None, 0:4, :].to_broadcast([128, n, 4, 3])
        linp = spool.tile([128, n, 4, 3], f32, name="linp" + suf, bufs=2)
        e = spool.tile([128, n, 4, 3], f32, name="e" + suf, bufs=2)
        ystg = bpool.tile([128, n, 4, 3], f32, name="ystg" + suf, bufs=2)
        PTT(out=linp[:], in0=hps[:, :, :, 0:3], in1=cb_lin, op=add)
        PTT(out=e[:], in0=hps[:, :, :, 3:6], in1=xrb[:, hs, :, 0:3],
            op=mult)
        PTT(out=e[:], in0=e[:], in1=xrb[:, hs, :, 6:9], op=add)
        PTT(out=e[:], in0=e[:], in1=linp[:], op=mult)
        PTT(out=ystg[:], in0=e[:], in1=xrb[:, hs, :, 3:6], op=add)
        nc.sync.dma_start(
            Y[:, bi * CPB + off:bi * CPB + off + n, :, :], ystg[:])

    # Software pipeline over chunks; at iteration ci every instruction's
    # inputs were produced in earlier iterations, so no engine waits on
    # another mid-period:
    #   PE : heads(ci-4), L1(ci), L2(ci-2)        (L2b emitted last)
    #   ACT: h2 castA(ci-3), castB(ci-3)
    #   DVE: h1 cast(ci-1) (+ hp copy at batch ends)
    #   Pool: epilogue arithmetic (SBUF only)
    # h2 PSUM ring-3: slot(L2a(c)) is freed by castA(c) early in iter c+3
    # and reused by L2b(c+1) late in the same iteration.
    stage_in(0)
    for ci in range(NCHUNK + 4):
        ring = {}
        cj0 = ci - 2
        if 0 <= cj0 < NCHUNK and "nol2" not in ABL:
            ring["h2pa"] = ps2.tile([128, CHUNK], f32, name="p2", bufs=3)
            ring["h2pb"] = ps2.tile([128, CHUNK], f32, name="p2", bufs=3)
        if ci < NCHUNK:
            ring["h1p"] = ps1.tile([128, 2, CHUNK], f32, name="p1", bufs=2)
        ck = ci - 4
        if ck >= 0 and "noheads" not in ABL:
            bi, cbk = divmod(ck, CPB)
            if cbk == 0:
                hp_t[bi] = pshead.tile([128, CPB, 4, 6], f32, name="hp",
                                       bufs=1)
            hp = hp_t[bi]
            h2s = h2s_t[ck]
            h2s_t[ck] = None
            # heads: hp[:, cbk, s, 0:3] = lin_mm, hp[:, cbk, s, 3:6] = p_mm
            # non-DR accumulating pairs: DR at FD=6 disables FWL and the
            # LDWEIGHTS overhead dwarfs the matmul
            for s in range(4):
                seg = slice(s * 128, (s + 1) * 128)
                nc.tensor.matmul(hp[:, cbk, s, :], h2s[:, 0, seg],
                                 w2h[:, 0, 256:262], start=True, stop=False)
                nc.tensor.matmul(hp[:, cbk, s, :], h2s[:, 1, seg],
                                 w2h[:, 1, 256:262], start=False, stop=True)

        ch = ci - 3
        if 0 <= ch < NCHUNK and "nocast2" not in ABL:
            # h2 casts on DVE: relu(h2 + 16*b2) per half, fp8 out at x16
            h2pa, h2pb = h2p_t[ch]
            h2p_t[ch] = None
            h2s = h2pool.tile([128, 2, CHUNK], f8, name="h2s", bufs=3)
            nc.vector.tensor_scalar(h2s[:, 0, :], h2pa[:], cb[:, 4, 0:1],
                                    0.0, op0=add, op1=amax)
            nc.vector.tensor_scalar(h2s[:, 1, :], h2pb[:], cb[:, 4, 1:2],
                                    0.0, op0=add, op1=amax)
            h2s_t[ch] = h2s

        if ci < NCHUNK:
            bi, cbk = divmod(ci, CPB)
            if cbk == 0 and bi + 1 < NBATCH:
                stage_in(bi + 1)
            # L1: H1T = W1T.T @ XT, bias via augmented row
            # weights host-scaled x64 for fp8; descaled in the DVE cast
            h1p = ring["h1p"]
            xt = xt_t[bi]
            sl = slice(cbk * CHUNK, (cbk + 1) * CHUNK)
            xsrc = xt_pre[:, 0:CHUNK] if ci == 0 else xt[:, sl]
            nc.tensor.matmul(h1p[:, 0, :], w1t[:, 0:128], xsrc,
                             start=True, stop=True)
            nc.tensor.matmul(h1p[:, 1, :], w1t[:, 128:256], xsrc,
                             start=True, stop=True)
            h1p_t[ci] = h1p

        ck1 = ci - 1
        if 0 <= ck1 < NCHUNK and "nocast1" not in ABL:
            # h1 cast on ACT: relu(h1p/64) in one [128,2,512] instr
            # (b1 folded via the augmented L1 row, so no bias needed)
            h1p = h1p_t[ck1]
            h1p_t[ck1] = None
            h1s = h1pool.tile([128, 2, CHUNK], f8, name="h1s",
                              bufs=H1SBUFS)
            nc.scalar.activation(h1s[:], h1p[:], Relu, scale=1.0 / 64)
            h1s_t[ck1] = h1s

        cj = ci - 2
        if 0 <= cj < NCHUNK and "nol2" not in ABL:
            # L2: H2T = W2T.T @ H1T, one DoubleRow matmul per half into
            # the PSUM ring (W2/B2 host-scaled x16)
            h1s = h1s_t[cj]
            h1s_t[cj] = None
            if h1s_static is not None:
                h1s = h1s_static
            h2pa = ring["h2pa"]
            h2pb = ring["h2pb"]
            # dead-store duplicates keep the PE duty cycle high so DVFS
            # holds max clock; the real L2a write lands last
            for _ in range(FILLER):
                nc.tensor.matmul(h2pa[:], w2h[:, :, 0:128], h1s[:],
                                 start=True, stop=True, perf_mode=DR)
            nc.tensor.matmul(h2pa[:], w2h[:, :, 0:128], h1s[:],
                             start=True, stop=True, perf_mode=DR)
            nc.tensor.matmul(h2pb[:], w2h[:, :, 128:256], h1s[:],
                             start=True, stop=True, perf_mode=DR)
            h2p_t[cj] = (h2pa, h2pb)

        # epilogue after a batch of heads completes; final batch split in
        # two half-batches to shorten the pipeline drain
        if ck >= 0 and "noepi" not in ABL:
            if ck == NCHUNK - 5:
                epilogue(NBATCH - 1, 0, CPB - 4, "q")
            elif ck == NCHUNK - 1:
                epilogue(NBATCH - 1, CPB - 4, 4, "q")
            elif ck % CPB == CPB - 1:
                epilogue(ck // CPB, 0, CPB, "")


def _get_nc():
    if "nc" not in _NC_CACHE:
        _NC_CACHE["nc"] = _build_nc()
    return _NC_CACHE["nc"]


def _prepare_inputs(inputs):
    X = np.asarray(inputs["X"], dtype=np.float32)
    W1 = np.asarray(inputs["W1"], dtype=np.float32)
    b1 = np.asarray(inputs["b1"], dtype=np.float32)
    W2 = np.asarray(inputs["W2"], dtype=np.float32)
    b2 = np.asarray(inputs["b2"], dtype=np.float32)
    W3 = np.asarray(inputs["W3"], dtype=np.float32)
    b3 = np.asarray(inputs["b3"], dtype=np.float32)

    lw, lb, bew, beb, bw, bb = {}, {}, {}, {}, {}, {}
    for t in ("incl", "ecc", "mm"):
        lw[t] = np.asarray(inputs[f"lin_w_{t}"], np.float32)[0]        # [128]
        lb[t] = float(np.asarray(inputs[f"lin_b_{t}"], np.float32)[0])
        bew[t] = np.asarray(inputs[f"bile_w_{t}"], np.float32)[0][:, 0]  # [128]
        beb[t] = float(np.asarray(inputs[f"bile_b_{t}"], np.float32)[0])
        bw[t] = float(np.asarray(inputs[f"bil_w_{t}"], np.float32)[0, 0, 0])
        bb[t] = float(np.asarray(inputs[f"bil_b_{t}"], np.float32)[0])
    TS = ("incl", "ecc", "mm")
    RIDX = {"incl": 7, "ecc": 9, "mm": 12}

    # ---- replicated weights (fp8 DoubleRow layouts) ----
    # scales: W1 x64 (descaled in h1 DVE cast), W2/B2 x16 (h2 lives at
    # 16x in fp8, max |h2|<15 assumed), HW2 x16 -> hp at 256x; the 1/256
    # descale is folded exactly (powers of 2) into CONSTS/XRB.
    F8 = ml_dtypes.float8_e4m3
    SC1, SC2, SCH = 64.0, 16.0, 16.0
    SP = SC2 * SCH                                                  # 256
    W1a = np.zeros((40, 256), np.float32)
    W1a[0:D] = W1.T * SC1
    W1a[D] = b1 * SC1
    W1T = np.ascontiguousarray(W1a).astype(F8)
    O6 = np.stack([lw[t] for t in TS] + [bew[t] for t in TS], axis=1)  # [128,6]
    HW2f = W3.T.astype(np.float32) @ O6                             # [256, 6]
    W2H = np.zeros((128, 2, 272), np.float32)
    # cols 0:256: W2.T [k, m] at [k % 128, k // 128, oh*128 + m], x16
    W2H[:, :, 0:256] = (
        W2.T.reshape(2, 128, 256).transpose(1, 0, 2)) * SC2
    W2H[:, :, 256:262] = (
        HW2f.reshape(2, 128, 6).transpose(1, 0, 2)) * SCH
    W2H = W2H.astype(F8)
    c3 = np.array(
        [lb[t] + float(b3 @ lw[t]) for t in TS],         # lb' (b3 folded)
        dtype=np.float32) * SP
    CB = np.empty((128, 5, 3), np.float32)
    CB[:, 0:4, :] = c3
    CB[:, 4, 0] = b2[0:128] * SC2
    CB[:, 4, 1] = b2[128:256] * SC2
    CB[:, 4, 2] = 0.0
    K = {t: float(b3 @ bew[t]) for t in TS}

    in_maps = []
    for c in range(NCORES):
        Xl = X[c * ROWS:(c + 1) * ROWS]                             # [32768, 38]
        XTf = np.zeros((40, ROWS), np.float32)
        XTf[0:D] = Xl.T
        XTf[D] = 1.0
        XTc = np.ascontiguousarray(XTf).astype(F8)
        eps = Xl[:, 0]
        E9 = np.empty((ROWS, 9), np.float32)
        for j, t in enumerate(TS):
            E9[:, j] = bw[t] * eps / (SP * SP)
            E9[:, 3 + j] = Xl[:, RIDX[t]] + bb[t]
            E9[:, 6 + j] = (bw[t] * beb[t] + (bw[t] * K[t]) * eps) / SP
        XRBc = np.ascontiguousarray(
            E9.reshape(NCHUNK, 4, 128, 9).transpose(2, 0, 1, 3))
        in_maps.append({
            "XT": XTc, "XRB": XRBc, "W1T": W1T, "W2H": W2H, "CB": CB,
        })
    return in_maps


def _gather_output(results):
    Y = np.empty((B, 3), np.float32)
    for c in range(NCORES):
        Ydev = np.asarray(results[c]["Y"], np.float32)   # [128, 64, 4, 3]
        Y[c * ROWS:(c + 1) * ROWS] = (
            Ydev.transpose(1, 2, 0, 3).reshape(ROWS, 3))
    return Y


def run(inputs, trace=False, **spmd_kwargs):
    from concourse import bass_utils

    nc = _get_nc()
    in_maps = _prepare_inputs(inputs)
    res = bass_utils.run_bass_kernel_spmd(
        nc, in_maps, list(range(NCORES)), trace=trace, **spmd_kwargs)
    return _gather_output(res.results), res


def kernel(**inputs):
    out, _ = run(inputs)
    return out

